# revision 1
# baseline (speedup 1.0000x reference)
"""GNN message-passing (2-layer conv + log_softmax) as a Bass/Tile SPMD kernel
on 8 Trainium2 NeuronCores.

Strategy (dst-sharded 1D graph partition, replicated message tables):
  - nodes sharded 8-way; core k owns dst nodes [k*NP, (k+1)*NP)
  - L1: h1 = x@W1 + b1 computed on node shards (host-pretransposed bf16 xT),
    chunk-wise AllGather -> full bf16 table tb1 (chunk-major row layout)
  - aggregation: per-core dsts sorted by in-degree, grouped into 128-dst
    windows padded to the window max degree; messages fetched with indirect
    DMA gathers (row per edge slot, pad slots hit a zero row) and reduced on
    the TensorEngine by identity-matmul PSUM accumulation (exact fp32)
  - elu folded as g' = relu(f) + exp(min(f,0)) = elu(f)+1, compensated by
    passing b2' = b2 - W2.sum(0); t2 = g'@W2 + b2' built per window (PE
    transpose + matmul), AllGather#2 -> table tb2; second gather+reduce;
    log_softmax fused on ACT/DVE. Output rows are in per-core degree-perm
    order; the host inverts the permutation.
"""

import os
import sys

sys.path.insert(0, "/opt/trn_rl_repo")

import numpy as np
import ml_dtypes

BF16 = ml_dtypes.bfloat16

# static problem config (full-size); tests may build their own cfg
N_CORES = 8
P = 128


def _make_cfg(n_nodes, n_edges, f_in=512, hid=64, n_cls=40, ctarget=256):
    np_ = n_nodes // N_CORES
    assert np_ * N_CORES == n_nodes
    nw = (np_ + P - 1) // P
    npad = nw * P
    n_chunks = min(4, nw)
    # chunk boundaries in units of 128-row tiles
    tiles = [nw // n_chunks + (1 if i < nw % n_chunks else 0) for i in range(n_chunks)]
    tstart = np.concatenate([[0], np.cumsum(tiles)])
    # table1 chunks cover real local rows [t0*128, min(t1*128, np_))
    c1_start = [int(min(tstart[i] * P, np_)) for i in range(n_chunks + 1)]
    c1_size = [c1_start[i + 1] - c1_start[i] for i in range(n_chunks)]
    # table2 chunks cover padded rows [t0*128, t1*128)
    c2_start = [int(tstart[i] * P) for i in range(n_chunks + 1)]
    c2_size = [c2_start[i + 1] - c2_start[i] for i in range(n_chunks)]
    base1 = np.concatenate([[0], np.cumsum([N_CORES * s for s in c1_size])])
    base2 = np.concatenate([[0], np.cumsum([N_CORES * s for s in c2_size])])
    tot1 = int(base1[-1])  # == n_nodes
    tot2 = int(base2[-1])  # == 8 * npad
    return dict(
        N=n_nodes, E=n_edges, F=f_in, H=hid, C=n_cls, NP=np_, NW=nw, NPAD=npad,
        NCH=n_chunks, TILES=tiles, TSTART=tstart,
        C1S=c1_start, C1Z=c1_size, C2S=c2_start, C2Z=c2_size,
        BASE1=base1, BASE2=base2, TOT1=tot1, TOT2=tot2,
        ZROW1=tot1, ZROW2=tot2, CTARGET=ctarget,
    )


FULL_CFG = _make_cfg(100000, 3200000)


# ---------------------------------------------------------------- host prep

def _row_maps(cfg, pos_all):
    """map global node id -> table1 row / table2 row (chunk-major layouts).
    pos_all: [N] position of each node within its core's degree-perm."""
    N, NP = cfg["N"], cfg["NP"]
    g = np.arange(N, dtype=np.int64)
    r = g // NP
    l = g % NP
    c1b = np.asarray(cfg["C1S"])
    c = np.searchsorted(c1b, l, side="right") - 1
    sz = np.asarray(cfg["C1Z"] + [1])[c]
    map1 = np.asarray(cfg["BASE1"])[c] + r * sz + (l - c1b[c])
    p = pos_all
    c2b = np.asarray(cfg["C2S"])
    c2 = np.searchsorted(c2b, p, side="right") - 1
    sz2 = np.asarray(cfg["C2Z"] + [1])[c2]
    map2 = np.asarray(cfg["BASE2"])[c2] + r * sz2 + (p - c2b[c2])
    map1 = np.concatenate([map1, [cfg["ZROW1"]]]).astype(np.int32)
    map2 = np.concatenate([map2, [cfg["ZROW2"]]]).astype(np.int32)
    return map1, map2


def host_prep(cfg, x, edge_index, W1, b1, W2, b2):
    N, NP, NW = cfg["N"], cfg["NP"], cfg["NW"]
    src = np.asarray(edge_index[0]).astype(np.int64)
    dst = np.asarray(edge_index[1]).astype(np.int64)

    per_core = []
    for k in range(N_CORES):
        sel = (dst >= k * NP) & (dst < (k + 1) * NP)
        s_k = src[sel]
        d_k = (dst[sel] - k * NP).astype(np.int64)
        deg = np.bincount(d_k, minlength=NP)
        perm = np.argsort(-deg, kind="stable")
        pos = np.empty(NP, dtype=np.int64)
        pos[perm] = np.arange(NP)
        order = np.argsort(d_k, kind="stable")
        ss = s_k[order]
        starts = np.concatenate([[0], np.cumsum(deg)])
        per_core.append(dict(deg=deg, perm=perm, pos=pos, ss=ss, starts=starts))

    # window capacities (uniform across cores)
    D = np.zeros(NW, dtype=np.int64)
    for k in range(N_CORES):
        deg, perm = per_core[k]["deg"], per_core[k]["perm"]
        for w in range(NW):
            n0 = perm[w * P] if w * P < NP else None
            dw = int(deg[n0]) if n0 is not None else 0
            D[w] = max(D[w], dw)
    D = np.maximum(D, 1)

    # greedy grouping of windows into gather calls
    groups = []  # (list of w, list of D_w, colstart)
    cur, curD = [], 0
    for w in range(NW):
        if cur and curD + D[w] > cfg["CTARGET"]:
            groups.append((cur, curD))
            cur, curD = [], 0
        cur.append(w)
        curD += int(D[w])
    if cur:
        groups.append((cur, curD))
    woff = np.concatenate([[0], np.cumsum(D)])  # col offset per window
    sumc = int(woff[-1])

    # raw src blocks per core (sentinel N for padding), then remap
    pos_all = np.concatenate([pc["pos"] for pc in per_core])
    map1, map2 = _row_maps(cfg, pos_all)
    idx1, idx2 = [], []
    for k in range(N_CORES):
        pc = per_core[k]
        raw = np.full((P, sumc), N, dtype=np.int64)
        deg, perm, ss, starts = pc["deg"], pc["perm"], pc["ss"], pc["starts"]
        for w in range(NW):
            for p in range(min(P, NP - w * P)):
                n = perm[w * P + p]
                dn = deg[n]
                if dn:
                    raw[p, woff[w]:woff[w] + dn] = ss[starts[n]:starts[n] + dn]
        idx1.append(map1[raw])
        idx2.append(map2[raw])

    # per-core tensors
    W1b = np.asarray(W1, dtype=np.float32).astype(BF16)
    W2b = np.asarray(W2, dtype=np.float32).astype(BF16)
    b1r = np.tile(np.asarray(b1, dtype=np.float32)[None, :], (P, 1))
    b2a = np.asarray(b2, dtype=np.float32) - np.asarray(W2, np.float32).sum(0)
    b2r = np.tile(b2a[None, :], (P, 1))
    in_maps = []
    xf = np.asarray(x, dtype=np.float32)
    for k in range(N_CORES):
        xT = np.ascontiguousarray(xf[k * NP:(k + 1) * NP].T).astype(BF16)
        in_maps.append(dict(
            xT=xT, W1=W1b, b1r=b1r, W2=W2b, b2r=b2r,
            idx1=idx1[k], idx2=idx2[k],
        ))
    sched = dict(D=D, groups=groups, woff=woff, sumc=sumc)
    perms = [pc["perm"] for pc in per_core]
    return sched, in_maps, perms


# ---------------------------------------------------------------- device code

def build_program(cfg, sched):
    import concourse.bass as bass
    import concourse.bacc as bacc
    import concourse.mybir as mybir
    from concourse.tile import TileContext
    from concourse.masks import make_identity

    dt = mybir.dt
    N, F, H, C = cfg["N"], cfg["F"], cfg["H"], cfg["C"]
    NP, NW, NPAD, NCH = cfg["NP"], cfg["NW"], cfg["NPAD"], cfg["NCH"]
    D, groups, woff, sumc = sched["D"], sched["groups"], sched["woff"], sched["sumc"]
    KF = F // P

    nc = bacc.Bacc(
        "TRN2", target_bir_lowering=False, debug=False, num_devices=N_CORES
    )
    xT = nc.declare_dram_parameter("xT", [F, NP], dt.bfloat16, isOutput=False)
    W1p = nc.declare_dram_parameter("W1", [F, H], dt.bfloat16, isOutput=False)
    b1p = nc.declare_dram_parameter("b1r", [P, H], dt.float32, isOutput=False)
    W2p = nc.declare_dram_parameter("W2", [H, C], dt.bfloat16, isOutput=False)
    b2p = nc.declare_dram_parameter("b2r", [P, C], dt.float32, isOutput=False)
    ix1p = nc.declare_dram_parameter("idx1", [P, sumc], dt.int32, isOutput=False)
    ix2p = nc.declare_dram_parameter("idx2", [P, sumc], dt.int32, isOutput=False)
    outp = nc.declare_dram_parameter("out", [NPAD, C], dt.float32, isOutput=True)

    rg = [list(range(N_CORES))]
    cmax = max(cD for _, cD in groups)

    with TileContext(nc) as tc:
        with (
            tc.tile_pool(name="const", bufs=1) as const,
            tc.tile_pool(name="dram", bufs=1, space="DRAM") as dram,
            tc.tile_pool(name="xp", bufs=3) as xp,
            tc.tile_pool(name="hp", bufs=3) as hp,
            tc.tile_pool(name="ixp", bufs=8) as ixp,
            tc.tile_pool(name="gp", bufs=8) as gpl,
            tc.tile_pool(name="sp", bufs=3) as sp,
            tc.tile_pool(name="ps", bufs=2, space="PSUM") as ps,
        ):
            # --- constants
            w1sb = const.tile([P, KF, H], dt.bfloat16)
            nc.sync.dma_start(out=w1sb[:], in_=W1p[:].rearrange("(c p) h -> p c h", p=P))
            w2sb = const.tile([H, C], dt.bfloat16)
            nc.sync.dma_start(out=w2sb[:], in_=W2p[:])
            b1sb = const.tile([P, H], dt.float32)
            nc.sync.dma_start(out=b1sb[:], in_=b1p[:])
            b2sb = const.tile([P, C], dt.float32)
            nc.sync.dma_start(out=b2sb[:], in_=b2p[:])
            ident = const.tile([P, P], dt.bfloat16)
            make_identity(nc, ident[:])

            # --- internal DRAM
            h1k = dram.tile([NP, H], dt.bfloat16)
            t2k = dram.tile([NPAD, C], dt.bfloat16)
            tb1 = dram.tile([cfg["TOT1"] + 1, H], dt.bfloat16)
            tb2 = dram.tile([cfg["TOT2"] + 1, C], dt.bfloat16)

            # zero rows for padding slots
            zt = const.tile([1, H], dt.bfloat16)
            nc.gpsimd.memset(zt[:], 0.0)
            nc.sync.dma_start(out=tb1[cfg["ZROW1"]:cfg["ZROW1"] + 1, :], in_=zt[:, :H])
            nc.sync.dma_start(out=tb2[cfg["ZROW2"]:cfg["ZROW2"] + 1, :], in_=zt[:, :C])

            # --- phase 1: h1 = x@W1 + b1 on local shard, chunked AllGather
            xTr = xT[:].rearrange("(c p) n -> p c n", p=P)
            for ch in range(NCH):
                t0, t1 = int(cfg["TSTART"][ch]), int(cfg["TSTART"][ch + 1])
                for nt in range(t0, t1):
                    cs = min(P, NP - nt * P)
                    if cs <= 0:
                        continue
                    xt = xp.tile([P, KF, P], dt.bfloat16, tag="xt")
                    nc.sync.dma_start(out=xt[:, :, :cs], in_=xTr[:, :, nt * P:nt * P + cs])
                    ph = ps.tile([P, H], dt.float32, tag="ph")
                    for kf in range(KF):
                        nc.tensor.matmul(
                            out=ph[:cs, :], lhsT=xt[:, kf, :cs], rhs=w1sb[:, kf, :],
                            start=(kf == 0), stop=(kf == KF - 1),
                        )
                    h1sb = hp.tile([P, H], dt.bfloat16, tag="h1sb")
                    nc.vector.tensor_tensor(
                        out=h1sb[:cs, :], in0=ph[:cs, :], in1=b1sb[:cs, :],
                        op=mybir.AluOpType.add,
                    )
                    nc.sync.dma_start(out=h1k[nt * P:nt * P + cs, :], in_=h1sb[:cs, :])
                # gather this chunk of h1 across cores
                s0, sz = cfg["C1S"][ch], cfg["C1Z"][ch]
                nc.gpsimd.collective_compute(
                    "AllGather", mybir.AluOpType.bypass, replica_groups=rg,
                    ins=[h1k[s0:s0 + sz, :]],
                    outs=[tb1[int(cfg["BASE1"][ch]):int(cfg["BASE1"][ch]) + N_CORES * sz, :]],
                )

            # --- phase 2: L1 gather+reduce, elu', t2 rows, chunked AllGather#2
            ch_end = {int(cfg["TSTART"][ch + 1]) - 1: ch for ch in range(NCH)}
            for w in range(NW):
                dw = int(D[w])
                c0 = int(woff[w])
                gts = []
                for s in range(dw):
                    ix = ixp.tile([P, 1], dt.int32, tag="ix")
                    nc.sync.dma_start(out=ix[:], in_=ix1p[:, c0 + s:c0 + s + 1])
                    gt = gpl.tile([P, H], dt.bfloat16, tag="gt")
                    nc.gpsimd.indirect_dma_start(
                        out=gt[:], out_offset=None,
                        in_=tb1[:],
                        in_offset=bass.IndirectOffsetOnAxis(ap=ix[:, :1], axis=0),
                    )
                    gts.append(gt)
                red = ps.tile([P, H], dt.float32, tag="red")
                for s in range(dw):
                    nc.tensor.matmul(
                        out=red[:, :], lhsT=ident[:],
                        rhs=gts[s][:],
                        start=(s == 0), stop=(s == dw - 1),
                    )
                if True:
                    # g' = relu(f) + exp(min(f, 0))  (= elu(f) + 1)
                    m = sp.tile([P, H], dt.float32, tag="m")
                    nc.vector.tensor_scalar_min(out=m[:], in0=red[:], scalar1=0.0)
                    e = sp.tile([P, H], dt.float32, tag="e")
                    nc.scalar.activation(e[:], m[:], mybir.ActivationFunctionType.Exp)
                    gpr = sp.tile([P, H], dt.bfloat16, tag="gpr")
                    nc.vector.scalar_tensor_tensor(
                        out=gpr[:], in0=red[:], scalar=0.0, in1=e[:],
                        op0=mybir.AluOpType.max, op1=mybir.AluOpType.add,
                    )
                    # t2 row block = g'@W2 + b2'
                    tr = ps.tile([H, P], dt.bfloat16, tag="tr")
                    nc.tensor.transpose(out=tr[:], in_=gpr[:], identity=ident[:])
                    trsb = sp.tile([H, P], dt.bfloat16, tag="trsb")
                    nc.vector.tensor_copy(out=trsb[:], in_=tr[:])
                    t2p = ps.tile([P, C], dt.float32, tag="t2p")
                    nc.tensor.matmul(out=t2p[:], lhsT=trsb[:], rhs=w2sb[:],
                                     start=True, stop=True)
                    t2sb = sp.tile([P, C], dt.bfloat16, tag="t2sb")
                    nc.vector.tensor_tensor(out=t2sb[:], in0=t2p[:], in1=b2sb[:, :C],
                                            op=mybir.AluOpType.add)
                    nc.sync.dma_start(out=t2k[w * P:(w + 1) * P, :], in_=t2sb[:])
                    if w in ch_end:
                        ch = ch_end[w]
                        s0, sz = cfg["C2S"][ch], cfg["C2Z"][ch]
                        nc.gpsimd.collective_compute(
                            "AllGather", mybir.AluOpType.bypass, replica_groups=rg,
                            ins=[t2k[s0:s0 + sz, :]],
                            outs=[tb2[int(cfg["BASE2"][ch]):int(cfg["BASE2"][ch]) + N_CORES * sz, :]],
                        )

            # --- phase 3: L2 gather+reduce + log_softmax
            for w in range(NW):
                dw = int(D[w])
                c0 = int(woff[w])
                gts = []
                for s in range(dw):
                    ix = ixp.tile([P, 1], dt.int32, tag="ix2")
                    nc.sync.dma_start(out=ix[:], in_=ix2p[:, c0 + s:c0 + s + 1])
                    gt = gpl.tile([P, C], dt.bfloat16, tag="gt2")
                    nc.gpsimd.indirect_dma_start(
                        out=gt[:], out_offset=None,
                        in_=tb2[:],
                        in_offset=bass.IndirectOffsetOnAxis(ap=ix[:, :1], axis=0),
                    )
                    gts.append(gt)
                red = ps.tile([P, C], dt.float32, tag="red")
                for s in range(dw):
                    nc.tensor.matmul(
                        out=red[:, :], lhsT=ident[:],
                        rhs=gts[s][:],
                        start=(s == 0), stop=(s == dw - 1),
                    )
                if True:
                    # log_softmax over classes
                    nm = sp.tile([P, 1], dt.float32, tag="nm")
                    nc.vector.tensor_reduce(
                        out=nm[:], in_=red[:], axis=mybir.AxisListType.X,
                        op=mybir.AluOpType.max, negate=True,
                    )
                    sc = sp.tile([P, C], dt.float32, tag="sc")
                    ssum = sp.tile([P, 1], dt.float32, tag="ssum")
                    nc.scalar.activation(
                        sc[:], red[:], mybir.ActivationFunctionType.Exp,
                        bias=nm[:], accum_out=ssum[:],
                    )
                    ls = sp.tile([P, 1], dt.float32, tag="ls")
                    nc.scalar.activation(ls[:], ssum[:], mybir.ActivationFunctionType.Ln)
                    ob = sp.tile([P, C], dt.float32, tag="ob")
                    nc.vector.tensor_scalar(
                        out=ob[:], in0=red[:], scalar1=nm[:], scalar2=ls[:],
                        op0=mybir.AluOpType.add, op1=mybir.AluOpType.subtract,
                    )
                    nc.sync.dma_start(out=outp[w * P:(w + 1) * P, :], in_=ob[:])

    nc.compile()
    return nc


# ---------------------------------------------------------------- entry point

LAST_RESULT = {}


def _run(cfg, x, edge_index, W1, b1, W2, b2, trace=False):
    from concourse.bass_utils import run_bass_kernel_spmd

    sched, in_maps, perms = host_prep(cfg, x, edge_index, W1, b1, W2, b2)
    nc = build_program(cfg, sched)
    res = run_bass_kernel_spmd(
        nc, in_maps, list(range(N_CORES)), trace=trace,
    )
    LAST_RESULT["exec_time_ns"] = res.exec_time_ns
    LAST_RESULT["mean_exec_time_ns"] = res.mean_exec_time_ns
    N, NP, C = cfg["N"], cfg["NP"], cfg["C"]
    full = np.empty((N, C), dtype=np.float32)
    for k in range(N_CORES):
        outk = np.asarray(res.results[k]["out"], dtype=np.float32)
        blk = full[k * NP:(k + 1) * NP]
        blk[perms[k]] = outk[:NP]
    return full


def kernel(x, edge_index, W1, b1, W2, b2):
    trace = bool(int(os.environ.get("GNN_TRACE", "0")))
    return _run(FULL_CFG, x, edge_index, W1, b1, W2, b2, trace=trace)



# revision 6
# speedup vs baseline: 1.9390x; 1.9390x over previous
"""GNN message-passing (2-layer conv + log_softmax) as a Bass/Tile SPMD kernel
on 8 Trainium2 NeuronCores.

Strategy (dst-sharded, dma_gather-based, 4 parallel SWDGE queues):
  - nodes sharded 8-way; core k owns dst nodes [k*NP, (k+1)*NP)
  - both conv layers aggregate 64-wide fp32 rows (W2 is applied AFTER the
    second aggregation -- conv2 is linear, so sum(elu(f1)[src])@W2 == result)
  - tables are fp32 [4*32768, 64] (256B rows, required by dma_gather) built
    by chunked AllGather; global node (j,l) maps to a section c (by l-chunk)
    at row 32768*c + j*sz[c] + (l - off[c]); idx into a section fits int16
  - per section: dsts sorted by per-section in-degree, 128-lane windows
    padded to the window max degree; slots fetched with dma_gather (256B
    elements, <=8192 idxs/call, round-robin over 4 SWDGE queues which
    generate descriptors in parallel); pad slots hit a zero row (32767)
  - per-window segment-sum on DVE (tensor_reduce over a strided 3D view),
    partials per section stored to DRAM, then combined per final window by
    4 more dma_gathers (idx = lane position of dst in each section's sort)
  - layer-1 final: f1 = sum of 4 partials; elu = relu(f)+exp(min(f,0))-1;
    rows written in natural dst order -> AllGather #2 -> table 2; layer 2
    reuses the SAME idx arrays (identical node->row mapping)
  - layer-2 final: agg2 @ W2 (PE transpose + matmul) + deg*b2, then
    log_softmax with a single batched Ln at the end. Output rows are in
    natural order; no host-side permutation.
"""

import os
import sys

sys.path.insert(0, "/opt/trn_rl_repo")

import numpy as np
import ml_dtypes

BF16 = ml_dtypes.bfloat16

N_CORES = 8
P = 128
SECCAP = 32768          # table rows per section
ZIDX = SECCAP - 1       # in-section idx of the zero row
NSEC = 4
MAXIDX = 8192           # max idxs per dma_gather call (SWDGE ring limit)


def _make_cfg(n_nodes, n_edges, f_in=512, hid=64, n_cls=40):
    np_ = n_nodes // N_CORES
    assert np_ * N_CORES == n_nodes
    nw = (np_ + P - 1) // P
    npad = nw * P
    q = min(4095, npad // 4)
    off = [0, q, 2 * q, 3 * q]
    sz = [q, q, q, npad - 3 * q]
    assert all(8 * s + 1 <= SECCAP for s in sz)
    return dict(
        N=n_nodes, E=n_edges, F=f_in, H=hid, C=n_cls,
        NP=np_, NW=nw, NPAD=npad, OFF=off, SZ=sz,
    )


FULL_CFG = _make_cfg(100000, 3200000)


# ---------------------------------------------------------------- host prep

def _map_rows(cfg, g):
    """global node id -> (section, in-section idx) of its table row."""
    NP = cfg["NP"]
    off = np.asarray(cfg["OFF"])
    sz = np.asarray(cfg["SZ"])
    j = g // NP
    l = g % NP
    c = np.searchsorted(off, l, side="right") - 1
    return c, j * sz[c] + (l - off[c])


def _wrap_idx(flat):
    """int16 flat idx list (len%16==0) -> [128, len//16] wrapped+replicated."""
    w = flat.reshape(-1, 16).T  # [16, n/16]
    return np.tile(w, (8, 1)).astype(np.int16)


def host_prep(cfg, x, edge_index, W1, b1, W2, b2):
    N, NP, NPAD, NW = cfg["N"], cfg["NP"], cfg["NPAD"], cfg["NW"]
    src = np.asarray(edge_index[0]).astype(np.int64)
    dst = np.asarray(edge_index[1]).astype(np.int64)
    ssec, sidx = _map_rows(cfg, src)  # per-edge section + in-section row idx

    # ---- per (core, section) degree tables
    deg = np.zeros((N_CORES, NSEC, NPAD), dtype=np.int64)
    core = dst // NP
    dl = dst % NP
    np.add.at(deg, (core, ssec, dl), 1)

    # window capacities D[c][w]: uniform across cores (max), min 1
    perms, poss = [], []  # per core per section
    D = np.ones((NSEC, NW), dtype=np.int64)
    for k in range(N_CORES):
        pk, qk = [], []
        for c in range(NSEC):
            pm = np.argsort(-deg[k, c], kind="stable")
            po = np.empty(NPAD, dtype=np.int64)
            po[pm] = np.arange(NPAD)
            pk.append(pm)
            qk.append(po)
            dw = deg[k, c][pm].reshape(NW, P).max(axis=1)
            D[c] = np.maximum(D[c], dw)
        perms.append(pk)
        poss.append(qk)

    # greedy call grouping per section: whole windows, sum(D) <= MAXIDX/128
    calls = []  # list of (section, w_lo, w_hi, n_idxs)
    woffD = np.zeros((NSEC, NW + 1), dtype=np.int64)
    for c in range(NSEC):
        woffD[c, 1:] = np.cumsum(D[c])
        wlo = 0
        while wlo < NW:
            whi = wlo + 1
            while whi < NW and (woffD[c, whi + 1] - woffD[c, wlo]) * P <= MAXIDX:
                whi += 1
            calls.append((c, wlo, whi, int((woffD[c, whi] - woffD[c, wlo]) * P)))
            wlo = whi
    # interleave calls across sections for queue balance
    by_sec = [[cl for cl in calls if cl[0] == c] for c in range(NSEC)]
    order = []
    i = 0
    while any(by_sec):
        for c in range(NSEC):
            if by_sec[c]:
                order.append(by_sec[c].pop(0))
        i += 1
    calls = order

    # ---- main gather idx arrays (per core): one flat array per section
    sec_tot = [int(woffD[c, NW]) for c in range(NSEC)]
    main_idx = []  # per core: per section flat int16 array [sec_tot[c]*128]
    for k in range(N_CORES):
        sel = core == k
        s_sec = ssec[sel]
        s_idx = sidx[sel]
        s_dl = dl[sel]
        per_sec = []
        for c in range(NSEC):
            m = s_sec == c
            lane = poss[k][c][s_dl[m]]
            o = np.lexsort((s_idx[m], lane))
            lane_s = lane[o]
            val_s = s_idx[m][o]
            # cumcount within lane
            startmask = np.ones(len(lane_s), dtype=bool)
            startmask[1:] = lane_s[1:] != lane_s[:-1]
            startpos = np.where(startmask, np.arange(len(lane_s)), 0)
            np.maximum.accumulate(startpos, out=startpos)
            slot = np.arange(len(lane_s)) - startpos
            w = lane_s // P
            p = lane_s % P
            arr = np.full(sec_tot[c] * P, ZIDX, dtype=np.int16)
            arr[(woffD[c, w] + slot) * P + p] = val_s.astype(np.int16)
            per_sec.append(arr)
        main_idx.append(per_sec)

    # pack main idx into one [128, TOT/16] tensor per core, call-ordered
    call_off = []  # col16 offset per call
    o16 = 0
    for (c, wlo, whi, n) in calls:
        call_off.append(o16)
        o16 += n // 16
    TOT16 = o16
    ixmain = np.zeros((N_CORES, P, TOT16), dtype=np.int16)
    for k in range(N_CORES):
        for j, (c, wlo, whi, n) in enumerate(calls):
            a = main_idx[k][c][woffD[c, wlo] * P: woffD[c, whi] * P]
            ixmain[k, :, call_off[j]: call_off[j] + n // 16] = _wrap_idx(a)

    # ---- combine gather idx: groups of up to 8 final windows, 4 sections
    NWG = 8
    groups = []
    w0 = 0
    while w0 < NW:
        groups.append((w0, min(w0 + NWG, NW)))
        w0 = min(w0 + NWG, NW)
    comb16 = sum((whi - wlo) * P * NSEC // 16 for (wlo, whi) in groups)
    ixcomb = np.zeros((N_CORES, P, comb16), dtype=np.int16)
    comb_off = []
    o16 = 0
    for (wlo, whi) in groups:
        offs = []
        for c in range(NSEC):
            offs.append(o16)
            o16 += (whi - wlo) * P // 16
        comb_off.append(offs)
    for k in range(N_CORES):
        for gi, (wlo, whi) in enumerate(groups):
            d = np.arange(wlo * P, whi * P)
            for c in range(NSEC):
                a = poss[k][c][d].astype(np.int16)
                ixcomb[k, :, comb_off[gi][c]: comb_off[gi][c] + len(a) // 16] = \
                    _wrap_idx(a)

    # ---- per-core tensors
    W1b = np.asarray(W1, dtype=np.float32).astype(BF16)
    W2b = np.asarray(W2, dtype=np.float32).astype(BF16)
    b1r = np.tile(np.asarray(b1, dtype=np.float32)[None, :], (P, 1))
    degtot = deg.sum(axis=1)  # [cores, NPAD]
    b2v = np.asarray(b2, dtype=np.float32)
    xf = np.asarray(x, dtype=np.float32)
    in_maps = []
    for k in range(N_CORES):
        xT = np.ascontiguousarray(xf[k * NP:(k + 1) * NP].T).astype(BF16)
        degb2 = (degtot[k][:, None] * b2v[None, :]).astype(np.float32)
        in_maps.append(dict(
            xT=xT, W1=W1b, b1r=b1r, W2=W2b, degb2=degb2,
            ixmain=ixmain[k], ixcomb=ixcomb[k],
        ))
    sched = dict(
        D=D, calls=calls, call_off=call_off, TOT16=TOT16, woffD=woffD,
        groups=groups, comb_off=comb_off, COMB16=comb16,
    )
    return sched, in_maps


# ---------------------------------------------------------------- device code

def build_program(cfg, sched):
    import concourse.bass as bass
    import concourse.bacc as bacc
    import concourse.mybir as mybir
    from concourse.tile import TileContext
    from concourse.masks import make_identity

    dt = mybir.dt
    F, H, C = cfg["F"], cfg["H"], cfg["C"]
    NP, NW, NPAD = cfg["NP"], cfg["NW"], cfg["NPAD"]
    OFF, SZ = cfg["OFF"], cfg["SZ"]
    D, calls, call_off = sched["D"], sched["calls"], sched["call_off"]
    woffD, groups, comb_off = sched["woffD"], sched["groups"], sched["comb_off"]
    KF = F // P
    NWG_MAX = max(whi - wlo for (wlo, whi) in groups)

    nc = bacc.Bacc(
        "TRN2", target_bir_lowering=False, debug=False, num_devices=N_CORES,
        num_swdge_queues=4,
    )
    xT = nc.declare_dram_parameter("xT", [F, NP], dt.bfloat16, isOutput=False)
    W1p = nc.declare_dram_parameter("W1", [F, H], dt.bfloat16, isOutput=False)
    b1p = nc.declare_dram_parameter("b1r", [P, H], dt.float32, isOutput=False)
    W2p = nc.declare_dram_parameter("W2", [H, C], dt.bfloat16, isOutput=False)
    dgb = nc.declare_dram_parameter("degb2", [NPAD, C], dt.float32, isOutput=False)
    ixm = nc.declare_dram_parameter("ixmain", [P, sched["TOT16"]], dt.int16,
                                    isOutput=False)
    ixc = nc.declare_dram_parameter("ixcomb", [P, sched["COMB16"]], dt.int16,
                                    isOutput=False)
    outp = nc.declare_dram_parameter("out", [NPAD, C], dt.float32, isOutput=True)

    rg = [list(range(N_CORES))]

    with TileContext(nc) as tc:
        with (
            tc.tile_pool(name="const", bufs=1) as const,
            tc.tile_pool(name="dram", bufs=1, space="DRAM") as dram,
            tc.tile_pool(name="xp", bufs=3) as xp,
            tc.tile_pool(name="hp", bufs=3) as hp,
            tc.tile_pool(name="ixp", bufs=4) as ixp,
            tc.tile_pool(name="gp", bufs=3) as gpl,
            tc.tile_pool(name="pw", bufs=6) as pw,
            tc.tile_pool(name="cb", bufs=2) as cb,
            tc.tile_pool(name="sp", bufs=2) as sp,
            tc.tile_pool(name="fin", bufs=1) as fin,
            tc.tile_pool(name="ps", bufs=2, space="PSUM") as ps,
        ):
            # --- constants
            w1sb = const.tile([P, KF, H], dt.bfloat16)
            nc.sync.dma_start(out=w1sb[:], in_=W1p[:].rearrange("(c p) h -> p c h", p=P))
            w2sb = const.tile([H, C], dt.bfloat16)
            nc.sync.dma_start(out=w2sb[:], in_=W2p[:])
            b1sb = const.tile([P, H], dt.float32)
            nc.sync.dma_start(out=b1sb[:], in_=b1p[:])
            ident = const.tile([P, P], dt.bfloat16)
            make_identity(nc, ident[:])
            zrow = const.tile([1, H], dt.float32)
            nc.gpsimd.memset(zrow[:], 0.0)

            # --- DRAM
            h1k = dram.tile([NPAD, H], dt.float32)
            t2k = dram.tile([NPAD, H], dt.float32)
            tbl1 = dram.tile([NSEC * SECCAP, H], dt.float32)
            tbl2 = dram.tile([NSEC * SECCAP, H], dt.float32)
            part1 = dram.tile([NSEC, NPAD, H], dt.float32)
            part2 = dram.tile([NSEC, NPAD, H], dt.float32)

            for tbl in (tbl1, tbl2):
                for c in range(NSEC):
                    nc.sync.dma_start(
                        out=tbl[c * SECCAP + ZIDX: c * SECCAP + ZIDX + 1, :],
                        in_=zrow[:],
                    )

            # --- phase 1: h1 = x@W1 + b1 (fp32 rows), chunked AllGather
            xTr = xT[:].rearrange("(c p) n -> p c n", p=P)
            for ch in range(NSEC):
                lo, hi = OFF[ch], OFF[ch] + SZ[ch]
                nt0, nt1 = lo // P, (hi + P - 1) // P
                for nt in range(nt0, nt1):
                    r0, r1 = max(lo, nt * P), min(hi, (nt + 1) * P, NP)
                    if r1 <= r0:
                        # rows beyond NP: nothing to compute (padding rows)
                        continue
                    cs = r1 - r0
                    xt = xp.tile([P, KF, P], dt.bfloat16, tag="xt")
                    nc.sync.dma_start(out=xt[:, :, :cs], in_=xTr[:, :, r0:r1])
                    ph = ps.tile([P, H], dt.float32, tag="ph")
                    for kf in range(KF):
                        nc.tensor.matmul(
                            out=ph[:cs, :], lhsT=xt[:, kf, :cs], rhs=w1sb[:, kf, :],
                            start=(kf == 0), stop=(kf == KF - 1),
                        )
                    h1sb = hp.tile([P, H], dt.float32, tag="h1sb")
                    nc.vector.tensor_tensor(
                        out=h1sb[:cs, :], in0=ph[:cs, :], in1=b1sb[:cs, :],
                        op=mybir.AluOpType.add,
                    )
                    nc.sync.dma_start(out=h1k[r0:r0 + cs, :], in_=h1sb[:cs, :])
                nc.gpsimd.collective_compute(
                    "AllGather", mybir.AluOpType.bypass, replica_groups=rg,
                    ins=[h1k[lo:hi, :]],
                    outs=[tbl1[ch * SECCAP: ch * SECCAP + N_CORES * SZ[ch], :]],
                )

            # --- aggregation machinery (used for both layers)
            def run_layer(tbl, part):
                # main gathers + per-window reduce -> partials
                stage = {}  # section -> (tile, w_start, count)
                qn = 0
                for j, (c, wlo, whi, n) in enumerate(calls):
                    B = n // P
                    ix = ixp.tile([P, MAXIDX // 16], dt.int16, tag="ix")
                    nc.sync.dma_start(
                        out=ix[:, : n // 16],
                        in_=ixm[:, call_off[j]: call_off[j] + n // 16],
                    )
                    gt = gpl.tile([P, MAXIDX // P, H], dt.float32, tag="gt")
                    nc.gpsimd.dma_gather(
                        out_ap=gt[:, :B, :],
                        in_ap=tbl[c * SECCAP: (c + 1) * SECCAP, :],
                        idxs_ap=ix[:, : n // 16],
                        num_idxs=n, num_idxs_reg=n, elem_size=H,
                        single_packet=False, queue_num=qn,
                    )
                    qn = (qn + 1) % 4
                    for w in range(wlo, whi):
                        b0 = int(woffD[c, w] - woffD[c, wlo])
                        dw = int(D[c, w])
                        if c not in stage or stage[c][1] + stage[c][2] != w \
                                or stage[c][2] == 8:
                            if c in stage:
                                _flush_stage(part, c, stage)
                            stage[c] = [pw.tile([P, 8, H], dt.float32, tag="st",
                                                name="st"),
                                        w, 0]
                        st, wst, cnt = stage[c]
                        nc.vector.tensor_reduce(
                            out=st[:, cnt, :],
                            in_=gt[:, b0:b0 + dw, :].rearrange("p b h -> p h b"),
                            axis=mybir.AxisListType.X,
                            op=mybir.AluOpType.add,
                        )
                        stage[c][2] += 1
                for c in list(stage.keys()):
                    _flush_stage(part, c, stage)

            def _flush_stage(part, c, stage):
                st, wst, cnt = stage.pop(c)
                nc.sync.dma_start(
                    out=part[c, wst * P:(wst + cnt) * P, :]
                        .rearrange("(w p) h -> p w h", p=P),
                    in_=st[:, :cnt, :],
                )

            def combine(part, gi):
                wlo, whi = groups[gi]
                nwg = whi - wlo
                big = cb.tile([P, NSEC, NWG_MAX, H], dt.float32, tag="big")
                for c in range(NSEC):
                    n = nwg * P
                    ix = ixp.tile([P, NWG_MAX * P // 16], dt.int16, tag="ixc")
                    nc.sync.dma_start(
                        out=ix[:, : n // 16],
                        in_=ixc[:, comb_off[gi][c]: comb_off[gi][c] + n // 16],
                    )
                    nc.gpsimd.dma_gather(
                        out_ap=big[:, c, :nwg, :],
                        in_ap=part[c],
                        idxs_ap=ix[:, : n // 16],
                        num_idxs=n, num_idxs_reg=n, elem_size=H,
                        single_packet=False, queue_num=c,
                    )
                t01 = sp.tile([P, NWG_MAX, H], dt.float32, tag="t01")
                nc.vector.tensor_tensor(out=t01[:, :nwg, :], in0=big[:, 0, :nwg, :],
                                        in1=big[:, 1, :nwg, :], op=mybir.AluOpType.add)
                t23 = sp.tile([P, NWG_MAX, H], dt.float32, tag="t23")
                nc.vector.tensor_tensor(out=t23[:, :nwg, :], in0=big[:, 2, :nwg, :],
                                        in1=big[:, 3, :nwg, :], op=mybir.AluOpType.add)
                f = sp.tile([P, NWG_MAX, H], dt.float32, tag="f")
                nc.vector.tensor_tensor(out=f[:, :nwg, :], in0=t01[:, :nwg, :],
                                        in1=t23[:, :nwg, :], op=mybir.AluOpType.add)
                return f, nwg

            # === layer 1 ===
            run_layer(tbl1, part1)

            # chunk-boundary bookkeeping for AllGather #2
            ch_after = {}
            for ch in range(NSEC):
                end = OFF[ch] + SZ[ch]
                for gi, (wlo, whi) in enumerate(groups):
                    if whi * P >= end:
                        ch_after.setdefault(gi, []).append(ch)
                        break

            for gi, (wlo, whi) in enumerate(groups):
                f, nwg = combine(part1, gi)
                # elu(f) = max(f,0) + exp(min(f,0)) - 1
                m = sp.tile([P, NWG_MAX, H], dt.float32, tag="m")
                nc.vector.tensor_scalar_min(out=m[:, :nwg, :], in0=f[:, :nwg, :],
                                            scalar1=0.0)
                e = sp.tile([P, NWG_MAX, H], dt.float32, tag="e")
                nc.scalar.activation(e[:, :nwg, :], m[:, :nwg, :],
                                     mybir.ActivationFunctionType.Exp)
                g1 = sp.tile([P, NWG_MAX, H], dt.float32, tag="g1")
                nc.vector.scalar_tensor_tensor(
                    out=g1[:, :nwg, :], in0=f[:, :nwg, :], scalar=0.0,
                    in1=e[:, :nwg, :],
                    op0=mybir.AluOpType.max, op1=mybir.AluOpType.add,
                )
                g2t = sp.tile([P, NWG_MAX, H], dt.float32, tag="g2t")
                nc.vector.tensor_scalar(
                    out=g2t[:, :nwg, :], in0=g1[:, :nwg, :], scalar1=-1.0,
                    scalar2=0.0, op0=mybir.AluOpType.add, op1=mybir.AluOpType.add,
                )
                nc.sync.dma_start(
                    out=t2k[wlo * P: whi * P, :].rearrange("(w p) h -> p w h", p=P),
                    in_=g2t[:, :nwg, :],
                )
                for ch in ch_after.get(gi, []):
                    lo, hi = OFF[ch], OFF[ch] + SZ[ch]
                    nc.gpsimd.collective_compute(
                        "AllGather", mybir.AluOpType.bypass, replica_groups=rg,
                        ins=[t2k[lo:hi, :]],
                        outs=[tbl2[ch * SECCAP: ch * SECCAP + N_CORES * SZ[ch], :]],
                    )

            # === layer 2 ===
            run_layer(tbl2, part2)

            t2f = fin.tile([P, NW, C], dt.float32)
            nmt = fin.tile([P, NW], dt.float32)
            sst = fin.tile([P, NW], dt.float32)
            for gi, (wlo, whi) in enumerate(groups):
                f, nwg = combine(part2, gi)
                db = sp.tile([P, NWG_MAX, C], dt.float32, tag="db")
                nc.sync.dma_start(
                    out=db[:, :nwg, :],
                    in_=dgb[wlo * P: whi * P, :].rearrange("(w p) c -> p w c", p=P),
                )
                for w in range(wlo, whi):
                    fb = sp.tile([P, H], dt.bfloat16, tag="fb")
                    nc.vector.tensor_copy(out=fb[:], in_=f[:, w - wlo, :])
                    tr = ps.tile([H, P], dt.bfloat16, tag="tr")
                    nc.tensor.transpose(out=tr[:], in_=fb[:], identity=ident[:])
                    trsb = sp.tile([H, P], dt.bfloat16, tag="trsb")
                    nc.vector.tensor_copy(out=trsb[:], in_=tr[:])
                    t2p = ps.tile([P, C], dt.float32, tag="t2p")
                    nc.tensor.matmul(out=t2p[:], lhsT=trsb[:], rhs=w2sb[:],
                                     start=True, stop=True)
                    nc.vector.tensor_tensor(
                        out=t2f[:, w, :], in0=t2p[:], in1=db[:, w - wlo, :],
                        op=mybir.AluOpType.add,
                    )
                    nc.vector.tensor_reduce(
                        out=nmt[:, w: w + 1], in_=t2f[:, w, :],
                        axis=mybir.AxisListType.X,
                        op=mybir.AluOpType.max, negate=True,
                    )
                    sc = sp.tile([P, C], dt.float32, tag="sc")
                    nc.scalar.activation(
                        sc[:], t2f[:, w, :], mybir.ActivationFunctionType.Exp,
                        bias=nmt[:, w: w + 1], accum_out=sst[:, w: w + 1],
                    )
            lnt = fin.tile([P, NW], dt.float32)
            nc.scalar.activation(lnt[:], sst[:], mybir.ActivationFunctionType.Ln)
            for gi, (wlo, whi) in enumerate(groups):
                nwg = whi - wlo
                ob = sp.tile([P, NWG_MAX, C], dt.float32, tag="ob")
                for w in range(wlo, whi):
                    nc.vector.tensor_scalar(
                        out=ob[:, w - wlo, :], in0=t2f[:, w, :],
                        scalar1=nmt[:, w: w + 1], scalar2=lnt[:, w: w + 1],
                        op0=mybir.AluOpType.add, op1=mybir.AluOpType.subtract,
                    )
                nc.sync.dma_start(
                    out=outp[wlo * P: whi * P, :].rearrange("(w p) c -> p w c", p=P),
                    in_=ob[:, :nwg, :],
                )

    nc.compile()
    return nc


# ---------------------------------------------------------------- entry point

LAST_RESULT = {}


def _run(cfg, x, edge_index, W1, b1, W2, b2, trace=False):
    from concourse.bass_utils import run_bass_kernel_spmd

    sched, in_maps = host_prep(cfg, x, edge_index, W1, b1, W2, b2)
    nc = build_program(cfg, sched)
    res = run_bass_kernel_spmd(nc, in_maps, list(range(N_CORES)), trace=trace)
    LAST_RESULT["exec_time_ns"] = res.exec_time_ns
    LAST_RESULT["mean_exec_time_ns"] = res.mean_exec_time_ns
    N, NP, C = cfg["N"], cfg["NP"], cfg["C"]
    full = np.empty((N, C), dtype=np.float32)
    for k in range(N_CORES):
        outk = np.asarray(res.results[k]["out"], dtype=np.float32)
        full[k * NP:(k + 1) * NP] = outk[:NP]
    return full


def kernel(x, edge_index, W1, b1, W2, b2):
    trace = bool(int(os.environ.get("GNN_TRACE", "0")))
    return _run(FULL_CFG, x, edge_index, W1, b1, W2, b2, trace=trace)


# revision 7
# speedup vs baseline: 2.5718x; 1.3264x over previous
"""GNN message-passing (2-layer conv + log_softmax) as a Bass/Tile SPMD kernel
on 8 Trainium2 NeuronCores.

Strategy (dst-sharded, dma_gather-based, 4 parallel SWDGE queues):
  - nodes sharded 8-way; core k owns dst nodes [k*NP, (k+1)*NP)
  - both conv layers aggregate 64-wide fp32 rows (W2 is applied AFTER the
    second aggregation -- conv2 is linear, so sum(elu(f1)[src])@W2 == result)
  - tables are fp32 [4*32768, 64] (256B rows, required by dma_gather) built
    by chunked AllGather; global node (j,l) maps to a section c (by l-chunk)
    at row 32768*c + j*sz[c] + (l - off[c]); idx into a section fits int16
  - per section: dsts sorted by per-section in-degree, 128-lane windows
    padded to the window max degree; slots fetched with dma_gather (256B
    elements, <=8192 idxs/call, round-robin over 4 SWDGE queues which
    generate descriptors in parallel); pad slots hit a zero row (32767)
  - per-window segment-sum on DVE (tensor_reduce over a strided 3D view),
    partials per section stored to DRAM, then combined per final window by
    4 more dma_gathers (idx = lane position of dst in each section's sort)
  - layer-1 final: f1 = sum of 4 partials; elu = relu(f)+exp(min(f,0))-1;
    rows written in natural dst order -> AllGather #2 -> table 2; layer 2
    reuses the SAME idx arrays (identical node->row mapping)
  - layer-2 final: agg2 @ W2 (PE transpose + matmul) + deg*b2, then
    log_softmax with a single batched Ln at the end. Output rows are in
    natural order; no host-side permutation.
"""

import os
import sys

sys.path.insert(0, "/opt/trn_rl_repo")

import numpy as np
import ml_dtypes

BF16 = ml_dtypes.bfloat16

N_CORES = 8
P = 128
SECCAP = 32768          # table rows per section
ZIDX = SECCAP - 1       # in-section idx of the zero row
NSEC = 4
MAXIDX = 4096           # max idxs per dma_gather call (SWDGE ring limit)


def _make_cfg(n_nodes, n_edges, f_in=512, hid=64, n_cls=40):
    np_ = n_nodes // N_CORES
    assert np_ * N_CORES == n_nodes
    nw = (np_ + P - 1) // P
    npad = nw * P
    q = min(4095, npad // 4)
    off = [0, q, 2 * q, 3 * q]
    sz = [q, q, q, npad - 3 * q]
    assert all(8 * s + 1 <= SECCAP for s in sz)
    return dict(
        N=n_nodes, E=n_edges, F=f_in, H=hid, C=n_cls,
        NP=np_, NW=nw, NPAD=npad, OFF=off, SZ=sz,
    )


FULL_CFG = _make_cfg(100000, 3200000)


# ---------------------------------------------------------------- host prep

def _map_rows(cfg, g):
    """global node id -> (section, in-section idx) of its table row."""
    NP = cfg["NP"]
    off = np.asarray(cfg["OFF"])
    sz = np.asarray(cfg["SZ"])
    j = g // NP
    l = g % NP
    c = np.searchsorted(off, l, side="right") - 1
    return c, j * sz[c] + (l - off[c])


def _wrap_idx(flat):
    """int16 flat idx list (len%16==0) -> [128, len//16] wrapped+replicated."""
    w = flat.reshape(-1, 16).T  # [16, n/16]
    return np.tile(w, (8, 1)).astype(np.int16)


def host_prep(cfg, x, edge_index, W1, b1, W2, b2):
    N, NP, NPAD, NW = cfg["N"], cfg["NP"], cfg["NPAD"], cfg["NW"]
    src = np.asarray(edge_index[0]).astype(np.int64)
    dst = np.asarray(edge_index[1]).astype(np.int64)
    ssec, sidx = _map_rows(cfg, src)  # per-edge section + in-section row idx

    # ---- per (core, section) degree tables
    deg = np.zeros((N_CORES, NSEC, NPAD), dtype=np.int64)
    core = dst // NP
    dl = dst % NP
    np.add.at(deg, (core, ssec, dl), 1)

    # window capacities D[c][w]: uniform across cores (max), min 1
    perms, poss = [], []  # per core per section
    D = np.ones((NSEC, NW), dtype=np.int64)
    for k in range(N_CORES):
        pk, qk = [], []
        for c in range(NSEC):
            pm = np.argsort(-deg[k, c], kind="stable")
            po = np.empty(NPAD, dtype=np.int64)
            po[pm] = np.arange(NPAD)
            pk.append(pm)
            qk.append(po)
            dw = deg[k, c][pm].reshape(NW, P).max(axis=1)
            D[c] = np.maximum(D[c], dw)
        perms.append(pk)
        poss.append(qk)

    # greedy call grouping per section: whole windows, sum(D) <= MAXIDX/128
    calls = []  # list of (section, w_lo, w_hi, n_idxs)
    woffD = np.zeros((NSEC, NW + 1), dtype=np.int64)
    for c in range(NSEC):
        woffD[c, 1:] = np.cumsum(D[c])
        wlo = 0
        while wlo < NW:
            whi = wlo + 1
            while whi < NW and (woffD[c, whi + 1] - woffD[c, wlo]) * P <= MAXIDX:
                whi += 1
            calls.append((c, wlo, whi, int((woffD[c, whi] - woffD[c, wlo]) * P)))
            wlo = whi
    # interleave calls across sections for queue balance
    by_sec = [[cl for cl in calls if cl[0] == c] for c in range(NSEC)]
    order = []
    i = 0
    while any(by_sec):
        for c in range(NSEC):
            if by_sec[c]:
                order.append(by_sec[c].pop(0))
        i += 1
    calls = order

    # ---- main gather idx arrays (per core): one flat array per section
    sec_tot = [int(woffD[c, NW]) for c in range(NSEC)]
    main_idx = []  # per core: per section flat int16 array [sec_tot[c]*128]
    for k in range(N_CORES):
        sel = core == k
        s_sec = ssec[sel]
        s_idx = sidx[sel]
        s_dl = dl[sel]
        per_sec = []
        for c in range(NSEC):
            m = s_sec == c
            lane = poss[k][c][s_dl[m]]
            o = np.lexsort((s_idx[m], lane))
            lane_s = lane[o]
            val_s = s_idx[m][o]
            # cumcount within lane
            startmask = np.ones(len(lane_s), dtype=bool)
            startmask[1:] = lane_s[1:] != lane_s[:-1]
            startpos = np.where(startmask, np.arange(len(lane_s)), 0)
            np.maximum.accumulate(startpos, out=startpos)
            slot = np.arange(len(lane_s)) - startpos
            w = lane_s // P
            p = lane_s % P
            arr = np.full(sec_tot[c] * P, ZIDX, dtype=np.int16)
            arr[(woffD[c, w] + slot) * P + p] = val_s.astype(np.int16)
            per_sec.append(arr)
        main_idx.append(per_sec)

    # pack main idx into one [128, TOT/16] tensor per core, call-ordered
    call_off = []  # col16 offset per call
    o16 = 0
    for (c, wlo, whi, n) in calls:
        call_off.append(o16)
        o16 += n // 16
    TOT16 = o16
    ixmain = np.zeros((N_CORES, P, TOT16), dtype=np.int16)
    for k in range(N_CORES):
        for j, (c, wlo, whi, n) in enumerate(calls):
            a = main_idx[k][c][woffD[c, wlo] * P: woffD[c, whi] * P]
            ixmain[k, :, call_off[j]: call_off[j] + n // 16] = _wrap_idx(a)

    # ---- combine gather idx: groups of up to 8 final windows, 4 sections
    NWG = 8
    groups = []
    w0 = 0
    while w0 < NW:
        groups.append((w0, min(w0 + NWG, NW)))
        w0 = min(w0 + NWG, NW)
    comb16 = sum((whi - wlo) * P * NSEC // 16 for (wlo, whi) in groups)
    ixcomb = np.zeros((N_CORES, P, comb16), dtype=np.int16)
    comb_off = []
    o16 = 0
    for (wlo, whi) in groups:
        offs = []
        for c in range(NSEC):
            offs.append(o16)
            o16 += (whi - wlo) * P // 16
        comb_off.append(offs)
    for k in range(N_CORES):
        for gi, (wlo, whi) in enumerate(groups):
            d = np.arange(wlo * P, whi * P)
            for c in range(NSEC):
                a = poss[k][c][d].astype(np.int16)
                ixcomb[k, :, comb_off[gi][c]: comb_off[gi][c] + len(a) // 16] = \
                    _wrap_idx(a)

    # ---- per-core tensors
    W1b = np.asarray(W1, dtype=np.float32).astype(BF16)
    W2b = np.asarray(W2, dtype=np.float32).astype(BF16)
    b1r = np.tile(np.asarray(b1, dtype=np.float32)[None, :], (P, 1))
    degtot = deg.sum(axis=1)  # [cores, NPAD]
    b2v = np.asarray(b2, dtype=np.float32)
    xf = np.asarray(x, dtype=np.float32)
    in_maps = []
    for k in range(N_CORES):
        xT = np.ascontiguousarray(xf[k * NP:(k + 1) * NP].T).astype(BF16)
        degb2 = (degtot[k][:, None] * b2v[None, :]).astype(np.float32)
        in_maps.append(dict(
            xT=xT, W1=W1b, b1r=b1r, W2=W2b, degb2=degb2,
            ixmain=ixmain[k], ixcomb=ixcomb[k],
        ))
    sched = dict(
        D=D, calls=calls, call_off=call_off, TOT16=TOT16, woffD=woffD,
        groups=groups, comb_off=comb_off, COMB16=comb16,
    )
    return sched, in_maps


# ---------------------------------------------------------------- device code

def build_program(cfg, sched):
    import concourse.bass as bass
    import concourse.bacc as bacc
    import concourse.mybir as mybir
    from concourse.tile import TileContext
    from concourse.masks import make_identity

    dt = mybir.dt
    F, H, C = cfg["F"], cfg["H"], cfg["C"]
    NP, NW, NPAD = cfg["NP"], cfg["NW"], cfg["NPAD"]
    OFF, SZ = cfg["OFF"], cfg["SZ"]
    D, calls, call_off = sched["D"], sched["calls"], sched["call_off"]
    woffD, groups, comb_off = sched["woffD"], sched["groups"], sched["comb_off"]
    KF = F // P
    NWG_MAX = max(whi - wlo for (wlo, whi) in groups)

    nc = bacc.Bacc(
        "TRN2", target_bir_lowering=False, debug=False, num_devices=N_CORES,
        num_swdge_queues=4,
    )
    xT = nc.declare_dram_parameter("xT", [F, NP], dt.bfloat16, isOutput=False)
    W1p = nc.declare_dram_parameter("W1", [F, H], dt.bfloat16, isOutput=False)
    b1p = nc.declare_dram_parameter("b1r", [P, H], dt.float32, isOutput=False)
    W2p = nc.declare_dram_parameter("W2", [H, C], dt.bfloat16, isOutput=False)
    dgb = nc.declare_dram_parameter("degb2", [NPAD, C], dt.float32, isOutput=False)
    ixm = nc.declare_dram_parameter("ixmain", [P, sched["TOT16"]], dt.int16,
                                    isOutput=False)
    ixc = nc.declare_dram_parameter("ixcomb", [P, sched["COMB16"]], dt.int16,
                                    isOutput=False)
    outp = nc.declare_dram_parameter("out", [NPAD, C], dt.float32, isOutput=True)

    rg = [list(range(N_CORES))]

    with TileContext(nc) as tc:
        with (
            tc.tile_pool(name="const", bufs=1) as const,
            tc.tile_pool(name="dram", bufs=1, space="DRAM") as dram,
            tc.tile_pool(name="xp", bufs=3) as xp,
            tc.tile_pool(name="hp", bufs=3) as hp,
            tc.tile_pool(name="ixp", bufs=6) as ixp,
            tc.tile_pool(name="gp", bufs=6) as gpl,
            tc.tile_pool(name="pw", bufs=6) as pw,
            tc.tile_pool(name="cb", bufs=2) as cb,
            tc.tile_pool(name="sp", bufs=2) as sp,
            tc.tile_pool(name="fin", bufs=1) as fin,
            tc.tile_pool(name="ps", bufs=2, space="PSUM") as ps,
        ):
            # --- constants
            w1sb = const.tile([P, KF, H], dt.bfloat16)
            nc.sync.dma_start(out=w1sb[:], in_=W1p[:].rearrange("(c p) h -> p c h", p=P))
            w2sb = const.tile([H, C], dt.bfloat16)
            nc.sync.dma_start(out=w2sb[:], in_=W2p[:])
            b1sb = const.tile([P, H], dt.float32)
            nc.sync.dma_start(out=b1sb[:], in_=b1p[:])
            ident = const.tile([P, P], dt.bfloat16)
            make_identity(nc, ident[:])
            zrow = const.tile([1, H], dt.float32)
            nc.gpsimd.memset(zrow[:], 0.0)

            # --- DRAM
            h1k = dram.tile([NPAD, H], dt.float32)
            t2k = dram.tile([NPAD, H], dt.float32)
            tbl1 = dram.tile([NSEC * SECCAP, H], dt.float32)
            tbl2 = dram.tile([NSEC * SECCAP, H], dt.float32)
            part1 = dram.tile([NSEC, NPAD, H], dt.float32)
            part2 = dram.tile([NSEC, NPAD, H], dt.float32)

            for tbl in (tbl1, tbl2):
                for c in range(NSEC):
                    nc.sync.dma_start(
                        out=tbl[c * SECCAP + ZIDX: c * SECCAP + ZIDX + 1, :],
                        in_=zrow[:],
                    )

            # --- phase 1: h1 = x@W1 + b1 (fp32 rows), chunked AllGather
            xTr = xT[:].rearrange("(c p) n -> p c n", p=P)
            for ch in range(NSEC):
                lo, hi = OFF[ch], OFF[ch] + SZ[ch]
                nt0, nt1 = lo // P, (hi + P - 1) // P
                for nt in range(nt0, nt1):
                    r0, r1 = max(lo, nt * P), min(hi, (nt + 1) * P, NP)
                    if r1 <= r0:
                        # rows beyond NP: nothing to compute (padding rows)
                        continue
                    cs = r1 - r0
                    xt = xp.tile([P, KF, P], dt.bfloat16, tag="xt")
                    nc.sync.dma_start(out=xt[:, :, :cs], in_=xTr[:, :, r0:r1])
                    ph = ps.tile([P, H], dt.float32, tag="ph")
                    for kf in range(KF):
                        nc.tensor.matmul(
                            out=ph[:cs, :], lhsT=xt[:, kf, :cs], rhs=w1sb[:, kf, :],
                            start=(kf == 0), stop=(kf == KF - 1),
                        )
                    h1sb = hp.tile([P, H], dt.float32, tag="h1sb")
                    nc.vector.tensor_tensor(
                        out=h1sb[:cs, :], in0=ph[:cs, :], in1=b1sb[:cs, :],
                        op=mybir.AluOpType.add,
                    )
                    nc.sync.dma_start(out=h1k[r0:r0 + cs, :], in_=h1sb[:cs, :])
                nc.gpsimd.collective_compute(
                    "AllGather", mybir.AluOpType.bypass, replica_groups=rg,
                    ins=[h1k[lo:hi, :]],
                    outs=[tbl1[ch * SECCAP: ch * SECCAP + N_CORES * SZ[ch], :]],
                )

            # --- aggregation machinery (used for both layers)
            def run_layer(tbl, part):
                # main gathers + per-window reduce -> partials
                stage = {}  # section -> (tile, w_start, count)
                qn = 0
                for j, (c, wlo, whi, n) in enumerate(calls):
                    B = n // P
                    ix = ixp.tile([P, MAXIDX // 16], dt.int16, tag="ix")
                    nc.sync.dma_start(
                        out=ix[:, : n // 16],
                        in_=ixm[:, call_off[j]: call_off[j] + n // 16],
                    )
                    gt = gpl.tile([P, MAXIDX // P, H], dt.float32, tag="gt")
                    nc.gpsimd.dma_gather(
                        out_ap=gt[:, :B, :],
                        in_ap=tbl[c * SECCAP: (c + 1) * SECCAP, :],
                        idxs_ap=ix[:, : n // 16],
                        num_idxs=n, num_idxs_reg=n, elem_size=H,
                        single_packet=False, queue_num=qn,
                    )
                    qn = (qn + 1) % 4
                    for w in range(wlo, whi):
                        b0 = int(woffD[c, w] - woffD[c, wlo])
                        dw = int(D[c, w])
                        if c not in stage or stage[c][1] + stage[c][2] != w \
                                or stage[c][2] == 8:
                            if c in stage:
                                _flush_stage(part, c, stage)
                            stage[c] = [pw.tile([P, 8, H], dt.float32, tag="st",
                                                name="st"),
                                        w, 0]
                        st, wst, cnt = stage[c]
                        nc.vector.tensor_reduce(
                            out=st[:, cnt, :],
                            in_=gt[:, b0:b0 + dw, :].rearrange("p b h -> p h b"),
                            axis=mybir.AxisListType.X,
                            op=mybir.AluOpType.add,
                        )
                        stage[c][2] += 1
                for c in list(stage.keys()):
                    _flush_stage(part, c, stage)

            def _flush_stage(part, c, stage):
                st, wst, cnt = stage.pop(c)
                nc.sync.dma_start(
                    out=part[c, wst * P:(wst + cnt) * P, :]
                        .rearrange("(w p) h -> p w h", p=P),
                    in_=st[:, :cnt, :],
                )

            def combine(part, gi):
                wlo, whi = groups[gi]
                nwg = whi - wlo
                big = cb.tile([P, NSEC, NWG_MAX, H], dt.float32, tag="big")
                for c in range(NSEC):
                    n = nwg * P
                    ix = ixp.tile([P, NWG_MAX * P // 16], dt.int16, tag="ixc")
                    nc.sync.dma_start(
                        out=ix[:, : n // 16],
                        in_=ixc[:, comb_off[gi][c]: comb_off[gi][c] + n // 16],
                    )
                    nc.gpsimd.dma_gather(
                        out_ap=big[:, c, :nwg, :],
                        in_ap=part[c],
                        idxs_ap=ix[:, : n // 16],
                        num_idxs=n, num_idxs_reg=n, elem_size=H,
                        single_packet=False, queue_num=c,
                    )
                t01 = sp.tile([P, NWG_MAX, H], dt.float32, tag="t01")
                nc.vector.tensor_tensor(out=t01[:, :nwg, :], in0=big[:, 0, :nwg, :],
                                        in1=big[:, 1, :nwg, :], op=mybir.AluOpType.add)
                t23 = sp.tile([P, NWG_MAX, H], dt.float32, tag="t23")
                nc.vector.tensor_tensor(out=t23[:, :nwg, :], in0=big[:, 2, :nwg, :],
                                        in1=big[:, 3, :nwg, :], op=mybir.AluOpType.add)
                f = sp.tile([P, NWG_MAX, H], dt.float32, tag="f")
                nc.vector.tensor_tensor(out=f[:, :nwg, :], in0=t01[:, :nwg, :],
                                        in1=t23[:, :nwg, :], op=mybir.AluOpType.add)
                return f, nwg

            # === layer 1 ===
            run_layer(tbl1, part1)

            # chunk-boundary bookkeeping for AllGather #2
            ch_after = {}
            for ch in range(NSEC):
                end = OFF[ch] + SZ[ch]
                for gi, (wlo, whi) in enumerate(groups):
                    if whi * P >= end:
                        ch_after.setdefault(gi, []).append(ch)
                        break

            for gi, (wlo, whi) in enumerate(groups):
                f, nwg = combine(part1, gi)
                # elu(f) = max(f,0) + exp(min(f,0)) - 1
                m = sp.tile([P, NWG_MAX, H], dt.float32, tag="m")
                nc.vector.tensor_scalar_min(out=m[:, :nwg, :], in0=f[:, :nwg, :],
                                            scalar1=0.0)
                e = sp.tile([P, NWG_MAX, H], dt.float32, tag="e")
                nc.scalar.activation(e[:, :nwg, :], m[:, :nwg, :],
                                     mybir.ActivationFunctionType.Exp)
                g1 = sp.tile([P, NWG_MAX, H], dt.float32, tag="g1")
                nc.vector.scalar_tensor_tensor(
                    out=g1[:, :nwg, :], in0=f[:, :nwg, :], scalar=0.0,
                    in1=e[:, :nwg, :],
                    op0=mybir.AluOpType.max, op1=mybir.AluOpType.add,
                )
                g2t = sp.tile([P, NWG_MAX, H], dt.float32, tag="g2t")
                nc.vector.tensor_scalar(
                    out=g2t[:, :nwg, :], in0=g1[:, :nwg, :], scalar1=-1.0,
                    scalar2=0.0, op0=mybir.AluOpType.add, op1=mybir.AluOpType.add,
                )
                nc.sync.dma_start(
                    out=t2k[wlo * P: whi * P, :].rearrange("(w p) h -> p w h", p=P),
                    in_=g2t[:, :nwg, :],
                )
                for ch in ch_after.get(gi, []):
                    lo, hi = OFF[ch], OFF[ch] + SZ[ch]
                    nc.gpsimd.collective_compute(
                        "AllGather", mybir.AluOpType.bypass, replica_groups=rg,
                        ins=[t2k[lo:hi, :]],
                        outs=[tbl2[ch * SECCAP: ch * SECCAP + N_CORES * SZ[ch], :]],
                    )

            # === layer 2 ===
            run_layer(tbl2, part2)

            t2f = fin.tile([P, NW, C], dt.float32)
            nmt = fin.tile([P, NW], dt.float32)
            sst = fin.tile([P, NW], dt.float32)
            for gi, (wlo, whi) in enumerate(groups):
                f, nwg = combine(part2, gi)
                db = sp.tile([P, NWG_MAX, C], dt.float32, tag="db")
                nc.sync.dma_start(
                    out=db[:, :nwg, :],
                    in_=dgb[wlo * P: whi * P, :].rearrange("(w p) c -> p w c", p=P),
                )
                for w in range(wlo, whi):
                    fb = sp.tile([P, H], dt.bfloat16, tag="fb")
                    nc.vector.tensor_copy(out=fb[:], in_=f[:, w - wlo, :])
                    tr = ps.tile([H, P], dt.bfloat16, tag="tr")
                    nc.tensor.transpose(out=tr[:], in_=fb[:], identity=ident[:])
                    trsb = sp.tile([H, P], dt.bfloat16, tag="trsb")
                    nc.vector.tensor_copy(out=trsb[:], in_=tr[:])
                    t2p = ps.tile([P, C], dt.float32, tag="t2p")
                    nc.tensor.matmul(out=t2p[:], lhsT=trsb[:], rhs=w2sb[:],
                                     start=True, stop=True)
                    nc.vector.tensor_tensor(
                        out=t2f[:, w, :], in0=t2p[:], in1=db[:, w - wlo, :],
                        op=mybir.AluOpType.add,
                    )
                    nc.vector.tensor_reduce(
                        out=nmt[:, w: w + 1], in_=t2f[:, w, :],
                        axis=mybir.AxisListType.X,
                        op=mybir.AluOpType.max, negate=True,
                    )
                    sc = sp.tile([P, C], dt.float32, tag="sc")
                    nc.scalar.activation(
                        sc[:], t2f[:, w, :], mybir.ActivationFunctionType.Exp,
                        bias=nmt[:, w: w + 1], accum_out=sst[:, w: w + 1],
                    )
            lnt = fin.tile([P, NW], dt.float32)
            nc.scalar.activation(lnt[:], sst[:], mybir.ActivationFunctionType.Ln)
            for gi, (wlo, whi) in enumerate(groups):
                nwg = whi - wlo
                ob = sp.tile([P, NWG_MAX, C], dt.float32, tag="ob")
                for w in range(wlo, whi):
                    nc.vector.tensor_scalar(
                        out=ob[:, w - wlo, :], in0=t2f[:, w, :],
                        scalar1=nmt[:, w: w + 1], scalar2=lnt[:, w: w + 1],
                        op0=mybir.AluOpType.add, op1=mybir.AluOpType.subtract,
                    )
                nc.sync.dma_start(
                    out=outp[wlo * P: whi * P, :].rearrange("(w p) c -> p w c", p=P),
                    in_=ob[:, :nwg, :],
                )

    nc.compile()
    return nc


# ---------------------------------------------------------------- entry point

LAST_RESULT = {}


def _run(cfg, x, edge_index, W1, b1, W2, b2, trace=False):
    from concourse.bass_utils import run_bass_kernel_spmd

    sched, in_maps = host_prep(cfg, x, edge_index, W1, b1, W2, b2)
    nc = build_program(cfg, sched)
    res = run_bass_kernel_spmd(nc, in_maps, list(range(N_CORES)), trace=trace)
    LAST_RESULT["exec_time_ns"] = res.exec_time_ns
    LAST_RESULT["mean_exec_time_ns"] = res.mean_exec_time_ns
    N, NP, C = cfg["N"], cfg["NP"], cfg["C"]
    full = np.empty((N, C), dtype=np.float32)
    for k in range(N_CORES):
        outk = np.asarray(res.results[k]["out"], dtype=np.float32)
        full[k * NP:(k + 1) * NP] = outk[:NP]
    return full


def kernel(x, edge_index, W1, b1, W2, b2):
    trace = bool(int(os.environ.get("GNN_TRACE", "0")))
    return _run(FULL_CFG, x, edge_index, W1, b1, W2, b2, trace=trace)


# revision 8
# speedup vs baseline: 2.6817x; 1.0427x over previous
"""GNN message-passing (2-layer conv + log_softmax) as a Bass/Tile SPMD kernel
on 8 Trainium2 NeuronCores.

Strategy (dst-sharded, dma_gather-based, 4 parallel SWDGE queues):
  - nodes sharded 8-way; core k owns dst nodes [k*NP, (k+1)*NP)
  - both conv layers aggregate 64-wide fp32 rows (W2 is applied AFTER the
    second aggregation -- conv2 is linear, so sum(elu(f1)[src])@W2 == result)
  - tables are fp32 [4*32768, 64] (256B rows, required by dma_gather) built
    by chunked AllGather; global node (j,l) maps to a section c (by l-chunk)
    at row 32768*c + j*sz[c] + (l - off[c]); idx into a section fits int16
  - per section: dsts sorted by per-section in-degree, 128-lane windows
    padded to the window max degree; slots fetched with dma_gather (256B
    elements, <=8192 idxs/call, round-robin over 4 SWDGE queues which
    generate descriptors in parallel); pad slots hit a zero row (32767)
  - per-window segment-sum on DVE (tensor_reduce over a strided 3D view),
    partials per section stored to DRAM, then combined per final window by
    4 more dma_gathers (idx = lane position of dst in each section's sort)
  - layer-1 final: f1 = sum of 4 partials; elu = relu(f)+exp(min(f,0))-1;
    rows written in natural dst order -> AllGather #2 -> table 2; layer 2
    reuses the SAME idx arrays (identical node->row mapping)
  - layer-2 final: agg2 @ W2 (PE transpose + matmul) + deg*b2, then
    log_softmax with a single batched Ln at the end. Output rows are in
    natural order; no host-side permutation.
"""

import os
import sys

sys.path.insert(0, "/opt/trn_rl_repo")

import numpy as np
import ml_dtypes

BF16 = ml_dtypes.bfloat16

N_CORES = 8
P = 128
SECCAP = 32768          # table rows per section
ZIDX = SECCAP - 1       # in-section idx of the zero row
NSEC = 4
MAXIDX = 4096           # max idxs per dma_gather call (SWDGE ring limit)


def _make_cfg(n_nodes, n_edges, f_in=512, hid=64, n_cls=40):
    np_ = n_nodes // N_CORES
    assert np_ * N_CORES == n_nodes
    nw = (np_ + P - 1) // P
    npad = nw * P
    q = min(4095, npad // 4)
    off = [0, q, 2 * q, 3 * q]
    sz = [q, q, q, npad - 3 * q]
    assert all(8 * s + 1 <= SECCAP for s in sz)
    return dict(
        N=n_nodes, E=n_edges, F=f_in, H=hid, C=n_cls,
        NP=np_, NW=nw, NPAD=npad, OFF=off, SZ=sz,
    )


FULL_CFG = _make_cfg(100000, 3200000)


# ---------------------------------------------------------------- host prep

def _map_rows(cfg, g):
    """global node id -> (section, in-section idx) of its table row."""
    NP = cfg["NP"]
    off = np.asarray(cfg["OFF"])
    sz = np.asarray(cfg["SZ"])
    j = g // NP
    l = g % NP
    c = np.searchsorted(off, l, side="right") - 1
    return c, j * sz[c] + (l - off[c])


def _wrap_idx(flat):
    """int16 flat idx list (len%16==0) -> [128, len//16] wrapped+replicated."""
    w = flat.reshape(-1, 16).T  # [16, n/16]
    return np.tile(w, (8, 1)).astype(np.int16)


def host_prep(cfg, x, edge_index, W1, b1, W2, b2):
    N, NP, NPAD, NW = cfg["N"], cfg["NP"], cfg["NPAD"], cfg["NW"]
    src = np.asarray(edge_index[0]).astype(np.int64)
    dst = np.asarray(edge_index[1]).astype(np.int64)
    ssec, sidx = _map_rows(cfg, src)  # per-edge section + in-section row idx

    # ---- per (core, section) degree tables
    deg = np.zeros((N_CORES, NSEC, NPAD), dtype=np.int64)
    core = dst // NP
    dl = dst % NP
    np.add.at(deg, (core, ssec, dl), 1)

    # window capacities D[c][w]: uniform across cores (max), min 1
    perms, poss = [], []  # per core per section
    D = np.ones((NSEC, NW), dtype=np.int64)
    for k in range(N_CORES):
        pk, qk = [], []
        for c in range(NSEC):
            pm = np.argsort(-deg[k, c], kind="stable")
            po = np.empty(NPAD, dtype=np.int64)
            po[pm] = np.arange(NPAD)
            pk.append(pm)
            qk.append(po)
            dw = deg[k, c][pm].reshape(NW, P).max(axis=1)
            D[c] = np.maximum(D[c], dw)
        perms.append(pk)
        poss.append(qk)

    # greedy call grouping per section: whole windows, sum(D) <= MAXIDX/128
    calls = []  # list of (section, w_lo, w_hi, n_idxs)
    woffD = np.zeros((NSEC, NW + 1), dtype=np.int64)
    for c in range(NSEC):
        woffD[c, 1:] = np.cumsum(D[c])
        wlo = 0
        while wlo < NW:
            whi = wlo + 1
            while whi < NW and (woffD[c, whi + 1] - woffD[c, wlo]) * P <= MAXIDX:
                whi += 1
            calls.append((c, wlo, whi, int((woffD[c, whi] - woffD[c, wlo]) * P)))
            wlo = whi
    # interleave calls across sections for queue balance
    by_sec = [[cl for cl in calls if cl[0] == c] for c in range(NSEC)]
    order = []
    i = 0
    while any(by_sec):
        for c in range(NSEC):
            if by_sec[c]:
                order.append(by_sec[c].pop(0))
        i += 1
    calls = order

    # ---- main gather idx arrays (per core): one flat array per section
    sec_tot = [int(woffD[c, NW]) for c in range(NSEC)]
    main_idx = []  # per core: per section flat int16 array [sec_tot[c]*128]
    for k in range(N_CORES):
        sel = core == k
        s_sec = ssec[sel]
        s_idx = sidx[sel]
        s_dl = dl[sel]
        per_sec = []
        for c in range(NSEC):
            m = s_sec == c
            lane = poss[k][c][s_dl[m]]
            o = np.lexsort((s_idx[m], lane))
            lane_s = lane[o]
            val_s = s_idx[m][o]
            # cumcount within lane
            startmask = np.ones(len(lane_s), dtype=bool)
            startmask[1:] = lane_s[1:] != lane_s[:-1]
            startpos = np.where(startmask, np.arange(len(lane_s)), 0)
            np.maximum.accumulate(startpos, out=startpos)
            slot = np.arange(len(lane_s)) - startpos
            w = lane_s // P
            p = lane_s % P
            arr = np.full(sec_tot[c] * P, ZIDX, dtype=np.int16)
            arr[(woffD[c, w] + slot) * P + p] = val_s.astype(np.int16)
            per_sec.append(arr)
        main_idx.append(per_sec)

    # pack main idx into one [128, TOT/16] tensor per core, call-ordered
    call_off = []  # col16 offset per call
    o16 = 0
    for (c, wlo, whi, n) in calls:
        call_off.append(o16)
        o16 += n // 16
    TOT16 = o16
    ixmain = np.zeros((N_CORES, P, TOT16), dtype=np.int16)
    for k in range(N_CORES):
        for j, (c, wlo, whi, n) in enumerate(calls):
            a = main_idx[k][c][woffD[c, wlo] * P: woffD[c, whi] * P]
            ixmain[k, :, call_off[j]: call_off[j] + n // 16] = _wrap_idx(a)

    # ---- combine gather idx: groups of up to 8 final windows, 4 sections
    NWG = 8
    groups = []
    w0 = 0
    while w0 < NW:
        groups.append((w0, min(w0 + NWG, NW)))
        w0 = min(w0 + NWG, NW)
    comb16 = sum((whi - wlo) * P * NSEC // 16 for (wlo, whi) in groups)
    ixcomb = np.zeros((N_CORES, P, comb16), dtype=np.int16)
    comb_off = []
    o16 = 0
    for (wlo, whi) in groups:
        offs = []
        for c in range(NSEC):
            offs.append(o16)
            o16 += (whi - wlo) * P // 16
        comb_off.append(offs)
    for k in range(N_CORES):
        for gi, (wlo, whi) in enumerate(groups):
            d = np.arange(wlo * P, whi * P)
            for c in range(NSEC):
                a = poss[k][c][d].astype(np.int16)
                ixcomb[k, :, comb_off[gi][c]: comb_off[gi][c] + len(a) // 16] = \
                    _wrap_idx(a)

    # ---- per-core tensors
    W1b = np.asarray(W1, dtype=np.float32).astype(BF16)
    W2b = np.asarray(W2, dtype=np.float32).astype(BF16)
    b1r = np.tile(np.asarray(b1, dtype=np.float32)[None, :], (P, 1))
    degtot = deg.sum(axis=1)  # [cores, NPAD]
    b2v = np.asarray(b2, dtype=np.float32)
    xf = np.asarray(x, dtype=np.float32)
    in_maps = []
    for k in range(N_CORES):
        xT = np.ascontiguousarray(xf[k * NP:(k + 1) * NP].T).astype(BF16)
        degb2 = (degtot[k][:, None] * b2v[None, :]).astype(np.float32)
        in_maps.append(dict(
            xT=xT, W1=W1b, b1r=b1r, W2=W2b, degb2=degb2,
            ixmain=ixmain[k], ixcomb=ixcomb[k],
        ))
    sched = dict(
        D=D, calls=calls, call_off=call_off, TOT16=TOT16, woffD=woffD,
        groups=groups, comb_off=comb_off, COMB16=comb16,
    )
    return sched, in_maps


# ---------------------------------------------------------------- device code

def build_program(cfg, sched):
    import concourse.bass as bass
    import concourse.bacc as bacc
    import concourse.mybir as mybir
    from concourse.tile import TileContext
    from concourse.masks import make_identity

    dt = mybir.dt
    F, H, C = cfg["F"], cfg["H"], cfg["C"]
    NP, NW, NPAD = cfg["NP"], cfg["NW"], cfg["NPAD"]
    OFF, SZ = cfg["OFF"], cfg["SZ"]
    D, calls, call_off = sched["D"], sched["calls"], sched["call_off"]
    woffD, groups, comb_off = sched["woffD"], sched["groups"], sched["comb_off"]
    KF = F // P
    NWG_MAX = max(whi - wlo for (wlo, whi) in groups)

    nc = bacc.Bacc(
        "TRN2", target_bir_lowering=False, debug=False, num_devices=N_CORES,
        num_swdge_queues=4,
    )
    xT = nc.declare_dram_parameter("xT", [F, NP], dt.bfloat16, isOutput=False)
    W1p = nc.declare_dram_parameter("W1", [F, H], dt.bfloat16, isOutput=False)
    b1p = nc.declare_dram_parameter("b1r", [P, H], dt.float32, isOutput=False)
    W2p = nc.declare_dram_parameter("W2", [H, C], dt.bfloat16, isOutput=False)
    dgb = nc.declare_dram_parameter("degb2", [NPAD, C], dt.float32, isOutput=False)
    ixm = nc.declare_dram_parameter("ixmain", [P, sched["TOT16"]], dt.int16,
                                    isOutput=False)
    ixc = nc.declare_dram_parameter("ixcomb", [P, sched["COMB16"]], dt.int16,
                                    isOutput=False)
    outp = nc.declare_dram_parameter("out", [NPAD, C], dt.float32, isOutput=True)

    rg = [list(range(N_CORES))]

    with TileContext(nc) as tc:
        with (
            tc.tile_pool(name="const", bufs=1) as const,
            tc.tile_pool(name="dram", bufs=1, space="DRAM") as dram,
            tc.tile_pool(name="xp", bufs=3) as xp,
            tc.tile_pool(name="hp", bufs=3) as hp,
            tc.tile_pool(name="ixp", bufs=6) as ixp,
            tc.tile_pool(name="gp", bufs=6) as gpl,
            tc.tile_pool(name="pw", bufs=6) as pw,
            tc.tile_pool(name="cb", bufs=2) as cb,
            tc.tile_pool(name="sp", bufs=2) as sp,
            tc.tile_pool(name="fin", bufs=1) as fin,
            tc.tile_pool(name="ps", bufs=2, space="PSUM") as ps,
        ):
            # --- constants
            w1sb = const.tile([P, KF, H], dt.bfloat16)
            nc.sync.dma_start(out=w1sb[:], in_=W1p[:].rearrange("(c p) h -> p c h", p=P))
            w2sb = const.tile([H, C], dt.bfloat16)
            nc.sync.dma_start(out=w2sb[:], in_=W2p[:])
            b1sb = const.tile([P, H], dt.float32)
            nc.sync.dma_start(out=b1sb[:], in_=b1p[:])
            ident = const.tile([P, P], dt.bfloat16)
            make_identity(nc, ident[:])
            zrow = const.tile([1, H], dt.float32)
            nc.gpsimd.memset(zrow[:], 0.0)

            # --- DRAM
            h1k = dram.tile([NPAD, H], dt.float32)
            t2k = dram.tile([NPAD, H], dt.float32)
            tbl1 = [dram.tile([SECCAP, H], dt.float32, name=f"tbl1_{c}", tag=f"tbl1_{c}")
                    for c in range(NSEC)]
            tbl2 = [dram.tile([SECCAP, H], dt.float32, name=f"tbl2_{c}", tag=f"tbl2_{c}")
                    for c in range(NSEC)]
            part1 = dram.tile([NSEC, NPAD, H], dt.float32)
            part2 = dram.tile([NSEC, NPAD, H], dt.float32)

            for tbl in (tbl1, tbl2):
                for c in range(NSEC):
                    nc.sync.dma_start(
                        out=tbl[c][ZIDX: ZIDX + 1, :],
                        in_=zrow[:],
                    )

            # --- phase 1: h1 = x@W1 + b1 (fp32 rows), chunked AllGather
            xTr = xT[:].rearrange("(c p) n -> p c n", p=P)
            for ch in range(NSEC):
                lo, hi = OFF[ch], OFF[ch] + SZ[ch]
                nt0, nt1 = lo // P, (hi + P - 1) // P
                for nt in range(nt0, nt1):
                    r0, r1 = max(lo, nt * P), min(hi, (nt + 1) * P, NP)
                    if r1 <= r0:
                        # rows beyond NP: nothing to compute (padding rows)
                        continue
                    cs = r1 - r0
                    xt = xp.tile([P, KF, P], dt.bfloat16, tag="xt")
                    nc.sync.dma_start(out=xt[:, :, :cs], in_=xTr[:, :, r0:r1])
                    ph = ps.tile([P, H], dt.float32, tag="ph")
                    for kf in range(KF):
                        nc.tensor.matmul(
                            out=ph[:cs, :], lhsT=xt[:, kf, :cs], rhs=w1sb[:, kf, :],
                            start=(kf == 0), stop=(kf == KF - 1),
                        )
                    h1sb = hp.tile([P, H], dt.float32, tag="h1sb")
                    nc.vector.tensor_tensor(
                        out=h1sb[:cs, :], in0=ph[:cs, :], in1=b1sb[:cs, :],
                        op=mybir.AluOpType.add,
                    )
                    nc.sync.dma_start(out=h1k[r0:r0 + cs, :], in_=h1sb[:cs, :])
                nc.gpsimd.collective_compute(
                    "AllGather", mybir.AluOpType.bypass, replica_groups=rg,
                    ins=[h1k[lo:hi, :]],
                    outs=[tbl1[ch][: N_CORES * SZ[ch], :]],
                )

            # --- aggregation machinery (used for both layers)
            def run_layer(tbl, part):
                # main gathers + per-window reduce -> partials
                stage = {}  # section -> (tile, w_start, count)
                qn = 0
                for j, (c, wlo, whi, n) in enumerate(calls):
                    B = n // P
                    ix = ixp.tile([P, MAXIDX // 16], dt.int16, tag="ix")
                    nc.sync.dma_start(
                        out=ix[:, : n // 16],
                        in_=ixm[:, call_off[j]: call_off[j] + n // 16],
                    )
                    gt = gpl.tile([P, MAXIDX // P, H], dt.float32, tag="gt")
                    nc.gpsimd.dma_gather(
                        out_ap=gt[:, :B, :],
                        in_ap=tbl[c][:],
                        idxs_ap=ix[:, : n // 16],
                        num_idxs=n, num_idxs_reg=n, elem_size=H,
                        single_packet=False, queue_num=qn,
                    )
                    qn = (qn + 1) % 4
                    for w in range(wlo, whi):
                        b0 = int(woffD[c, w] - woffD[c, wlo])
                        dw = int(D[c, w])
                        if c not in stage or stage[c][1] + stage[c][2] != w \
                                or stage[c][2] == 8:
                            if c in stage:
                                _flush_stage(part, c, stage)
                            stage[c] = [pw.tile([P, 8, H], dt.float32, tag="st",
                                                name="st"),
                                        w, 0]
                        st, wst, cnt = stage[c]
                        nc.vector.tensor_reduce(
                            out=st[:, cnt, :],
                            in_=gt[:, b0:b0 + dw, :].rearrange("p b h -> p h b"),
                            axis=mybir.AxisListType.X,
                            op=mybir.AluOpType.add,
                        )
                        stage[c][2] += 1
                for c in list(stage.keys()):
                    _flush_stage(part, c, stage)

            def _flush_stage(part, c, stage):
                st, wst, cnt = stage.pop(c)
                nc.sync.dma_start(
                    out=part[c, wst * P:(wst + cnt) * P, :]
                        .rearrange("(w p) h -> p w h", p=P),
                    in_=st[:, :cnt, :],
                )

            def combine(part, gi):
                wlo, whi = groups[gi]
                nwg = whi - wlo
                big = cb.tile([P, NSEC, NWG_MAX, H], dt.float32, tag="big")
                for c in range(NSEC):
                    n = nwg * P
                    ix = ixp.tile([P, NWG_MAX * P // 16], dt.int16, tag="ixc")
                    nc.sync.dma_start(
                        out=ix[:, : n // 16],
                        in_=ixc[:, comb_off[gi][c]: comb_off[gi][c] + n // 16],
                    )
                    nc.gpsimd.dma_gather(
                        out_ap=big[:, c, :nwg, :],
                        in_ap=part[c],
                        idxs_ap=ix[:, : n // 16],
                        num_idxs=n, num_idxs_reg=n, elem_size=H,
                        single_packet=False, queue_num=c,
                    )
                t01 = sp.tile([P, NWG_MAX, H], dt.float32, tag="t01")
                nc.vector.tensor_tensor(out=t01[:, :nwg, :], in0=big[:, 0, :nwg, :],
                                        in1=big[:, 1, :nwg, :], op=mybir.AluOpType.add)
                t23 = sp.tile([P, NWG_MAX, H], dt.float32, tag="t23")
                nc.vector.tensor_tensor(out=t23[:, :nwg, :], in0=big[:, 2, :nwg, :],
                                        in1=big[:, 3, :nwg, :], op=mybir.AluOpType.add)
                f = sp.tile([P, NWG_MAX, H], dt.float32, tag="f")
                nc.vector.tensor_tensor(out=f[:, :nwg, :], in0=t01[:, :nwg, :],
                                        in1=t23[:, :nwg, :], op=mybir.AluOpType.add)
                return f, nwg

            # === layer 1 ===
            run_layer(tbl1, part1)

            # chunk-boundary bookkeeping for AllGather #2
            ch_after = {}
            for ch in range(NSEC):
                end = OFF[ch] + SZ[ch]
                for gi, (wlo, whi) in enumerate(groups):
                    if whi * P >= end:
                        ch_after.setdefault(gi, []).append(ch)
                        break

            for gi, (wlo, whi) in enumerate(groups):
                f, nwg = combine(part1, gi)
                # elu(f) = max(f,0) + exp(min(f,0)) - 1
                m = sp.tile([P, NWG_MAX, H], dt.float32, tag="m")
                nc.vector.tensor_scalar_min(out=m[:, :nwg, :], in0=f[:, :nwg, :],
                                            scalar1=0.0)
                e = sp.tile([P, NWG_MAX, H], dt.float32, tag="e")
                nc.scalar.activation(e[:, :nwg, :], m[:, :nwg, :],
                                     mybir.ActivationFunctionType.Exp)
                g1 = sp.tile([P, NWG_MAX, H], dt.float32, tag="g1")
                nc.vector.scalar_tensor_tensor(
                    out=g1[:, :nwg, :], in0=f[:, :nwg, :], scalar=0.0,
                    in1=e[:, :nwg, :],
                    op0=mybir.AluOpType.max, op1=mybir.AluOpType.add,
                )
                g2t = sp.tile([P, NWG_MAX, H], dt.float32, tag="g2t")
                nc.vector.tensor_scalar(
                    out=g2t[:, :nwg, :], in0=g1[:, :nwg, :], scalar1=-1.0,
                    scalar2=0.0, op0=mybir.AluOpType.add, op1=mybir.AluOpType.add,
                )
                nc.sync.dma_start(
                    out=t2k[wlo * P: whi * P, :].rearrange("(w p) h -> p w h", p=P),
                    in_=g2t[:, :nwg, :],
                )
                for ch in ch_after.get(gi, []):
                    lo, hi = OFF[ch], OFF[ch] + SZ[ch]
                    nc.gpsimd.collective_compute(
                        "AllGather", mybir.AluOpType.bypass, replica_groups=rg,
                        ins=[t2k[lo:hi, :]],
                        outs=[tbl2[ch][: N_CORES * SZ[ch], :]],
                    )

            # === layer 2 ===
            run_layer(tbl2, part2)

            t2f = fin.tile([P, NW, C], dt.float32)
            nmt = fin.tile([P, NW], dt.float32)
            sst = fin.tile([P, NW], dt.float32)
            for gi, (wlo, whi) in enumerate(groups):
                f, nwg = combine(part2, gi)
                db = sp.tile([P, NWG_MAX, C], dt.float32, tag="db")
                nc.sync.dma_start(
                    out=db[:, :nwg, :],
                    in_=dgb[wlo * P: whi * P, :].rearrange("(w p) c -> p w c", p=P),
                )
                for w in range(wlo, whi):
                    fb = sp.tile([P, H], dt.bfloat16, tag="fb")
                    nc.vector.tensor_copy(out=fb[:], in_=f[:, w - wlo, :])
                    tr = ps.tile([H, P], dt.bfloat16, tag="tr")
                    nc.tensor.transpose(out=tr[:], in_=fb[:], identity=ident[:])
                    trsb = sp.tile([H, P], dt.bfloat16, tag="trsb")
                    nc.vector.tensor_copy(out=trsb[:], in_=tr[:])
                    t2p = ps.tile([P, C], dt.float32, tag="t2p")
                    nc.tensor.matmul(out=t2p[:], lhsT=trsb[:], rhs=w2sb[:],
                                     start=True, stop=True)
                    nc.vector.tensor_tensor(
                        out=t2f[:, w, :], in0=t2p[:], in1=db[:, w - wlo, :],
                        op=mybir.AluOpType.add,
                    )
                    nc.vector.tensor_reduce(
                        out=nmt[:, w: w + 1], in_=t2f[:, w, :],
                        axis=mybir.AxisListType.X,
                        op=mybir.AluOpType.max, negate=True,
                    )
                    sc = sp.tile([P, C], dt.float32, tag="sc")
                    nc.scalar.activation(
                        sc[:], t2f[:, w, :], mybir.ActivationFunctionType.Exp,
                        bias=nmt[:, w: w + 1], accum_out=sst[:, w: w + 1],
                    )
            lnt = fin.tile([P, NW], dt.float32)
            nc.scalar.activation(lnt[:], sst[:], mybir.ActivationFunctionType.Ln)
            for gi, (wlo, whi) in enumerate(groups):
                nwg = whi - wlo
                ob = sp.tile([P, NWG_MAX, C], dt.float32, tag="ob")
                for w in range(wlo, whi):
                    nc.vector.tensor_scalar(
                        out=ob[:, w - wlo, :], in0=t2f[:, w, :],
                        scalar1=nmt[:, w: w + 1], scalar2=lnt[:, w: w + 1],
                        op0=mybir.AluOpType.add, op1=mybir.AluOpType.subtract,
                    )
                nc.sync.dma_start(
                    out=outp[wlo * P: whi * P, :].rearrange("(w p) c -> p w c", p=P),
                    in_=ob[:, :nwg, :],
                )

    nc.compile()
    return nc


# ---------------------------------------------------------------- entry point

LAST_RESULT = {}


def _run(cfg, x, edge_index, W1, b1, W2, b2, trace=False):
    from concourse.bass_utils import run_bass_kernel_spmd

    sched, in_maps = host_prep(cfg, x, edge_index, W1, b1, W2, b2)
    nc = build_program(cfg, sched)
    res = run_bass_kernel_spmd(nc, in_maps, list(range(N_CORES)), trace=trace)
    LAST_RESULT["exec_time_ns"] = res.exec_time_ns
    LAST_RESULT["mean_exec_time_ns"] = res.mean_exec_time_ns
    N, NP, C = cfg["N"], cfg["NP"], cfg["C"]
    full = np.empty((N, C), dtype=np.float32)
    for k in range(N_CORES):
        outk = np.asarray(res.results[k]["out"], dtype=np.float32)
        full[k * NP:(k + 1) * NP] = outk[:NP]
    return full


def kernel(x, edge_index, W1, b1, W2, b2):
    trace = bool(int(os.environ.get("GNN_TRACE", "0")))
    return _run(FULL_CFG, x, edge_index, W1, b1, W2, b2, trace=trace)


# revision 9
# speedup vs baseline: 2.9478x; 1.0993x over previous
"""GNN message-passing (2-layer conv + log_softmax) as a Bass/Tile SPMD kernel
on 8 Trainium2 NeuronCores.

Strategy (dst-sharded, dma_gather-based, 4 parallel SWDGE queues):
  - nodes sharded 8-way; core k owns dst nodes [k*NP, (k+1)*NP)
  - both conv layers aggregate 64-wide fp32 rows (W2 is applied AFTER the
    second aggregation -- conv2 is linear, so sum(elu(f1)[src])@W2 == result)
  - tables are fp32 [4*32768, 64] (256B rows, required by dma_gather) built
    by chunked AllGather; global node (j,l) maps to a section c (by l-chunk)
    at row 32768*c + j*sz[c] + (l - off[c]); idx into a section fits int16
  - per section: dsts sorted by per-section in-degree, 128-lane windows
    padded to the window max degree; slots fetched with dma_gather (256B
    elements, <=8192 idxs/call, round-robin over 4 SWDGE queues which
    generate descriptors in parallel); pad slots hit a zero row (32767)
  - per-window segment-sum on DVE (tensor_reduce over a strided 3D view),
    partials per section stored to DRAM, then combined per final window by
    4 more dma_gathers (idx = lane position of dst in each section's sort)
  - layer-1 final: f1 = sum of 4 partials; elu = relu(f)+exp(min(f,0))-1;
    rows written in natural dst order -> AllGather #2 -> table 2; layer 2
    reuses the SAME idx arrays (identical node->row mapping)
  - layer-2 final: agg2 @ W2 (PE transpose + matmul) + deg*b2, then
    log_softmax with a single batched Ln at the end. Output rows are in
    natural order; no host-side permutation.
"""

import os
import sys

sys.path.insert(0, "/opt/trn_rl_repo")

import numpy as np
import ml_dtypes

BF16 = ml_dtypes.bfloat16

N_CORES = 8
P = 128
SECCAP = 32768          # table rows per section
ZIDX = SECCAP - 1       # in-section idx of the zero row
NSEC = 4
MAXIDX = 4096           # max idxs per dma_gather call (SWDGE ring limit)


def _make_cfg(n_nodes, n_edges, f_in=512, hid=64, n_cls=40):
    np_ = n_nodes // N_CORES
    assert np_ * N_CORES == n_nodes
    nw = (np_ + P - 1) // P
    npad = nw * P
    q = min(4095, npad // 4)
    off = [0, q, 2 * q, 3 * q]
    sz = [q, q, q, npad - 3 * q]
    assert all(8 * s + 1 <= SECCAP for s in sz)
    return dict(
        N=n_nodes, E=n_edges, F=f_in, H=hid, C=n_cls,
        NP=np_, NW=nw, NPAD=npad, OFF=off, SZ=sz,
    )


FULL_CFG = _make_cfg(100000, 3200000)


# ---------------------------------------------------------------- host prep

def _map_rows(cfg, g):
    """global node id -> (section, in-section idx) of its table row."""
    NP = cfg["NP"]
    off = np.asarray(cfg["OFF"])
    sz = np.asarray(cfg["SZ"])
    j = g // NP
    l = g % NP
    c = np.searchsorted(off, l, side="right") - 1
    return c, j * sz[c] + (l - off[c])


def _wrap_idx(flat):
    """int16 flat idx list (len%16==0) -> [128, len//16] wrapped+replicated."""
    w = flat.reshape(-1, 16).T  # [16, n/16]
    return np.tile(w, (8, 1)).astype(np.int16)


def host_prep(cfg, x, edge_index, W1, b1, W2, b2):
    N, NP, NPAD, NW = cfg["N"], cfg["NP"], cfg["NPAD"], cfg["NW"]
    src = np.asarray(edge_index[0]).astype(np.int64)
    dst = np.asarray(edge_index[1]).astype(np.int64)
    ssec, sidx = _map_rows(cfg, src)  # per-edge section + in-section row idx

    # ---- per (core, section) degree tables
    deg = np.zeros((N_CORES, NSEC, NPAD), dtype=np.int64)
    core = dst // NP
    dl = dst % NP
    np.add.at(deg, (core, ssec, dl), 1)

    # window capacities D[c][w]: uniform across cores (max), min 1
    perms, poss = [], []  # per core per section
    D = np.ones((NSEC, NW), dtype=np.int64)
    for k in range(N_CORES):
        pk, qk = [], []
        for c in range(NSEC):
            pm = np.argsort(-deg[k, c], kind="stable")
            po = np.empty(NPAD, dtype=np.int64)
            po[pm] = np.arange(NPAD)
            pk.append(pm)
            qk.append(po)
            dw = deg[k, c][pm].reshape(NW, P).max(axis=1)
            D[c] = np.maximum(D[c], dw)
        perms.append(pk)
        poss.append(qk)

    # greedy call grouping per section: whole windows, sum(D) <= MAXIDX/128
    calls = []  # list of (section, w_lo, w_hi, n_idxs)
    woffD = np.zeros((NSEC, NW + 1), dtype=np.int64)
    for c in range(NSEC):
        woffD[c, 1:] = np.cumsum(D[c])
        wlo = 0
        while wlo < NW:
            whi = wlo + 1
            while whi < NW and (woffD[c, whi + 1] - woffD[c, wlo]) * P <= MAXIDX:
                whi += 1
            calls.append((c, wlo, whi, int((woffD[c, whi] - woffD[c, wlo]) * P)))
            wlo = whi
    # section-major order: all queues start on section 0 while later
    # sections' AllGather chunks are still in flight

    # ---- main gather idx arrays (per core): one flat array per section
    sec_tot = [int(woffD[c, NW]) for c in range(NSEC)]
    main_idx = []  # per core: per section flat int16 array [sec_tot[c]*128]
    for k in range(N_CORES):
        sel = core == k
        s_sec = ssec[sel]
        s_idx = sidx[sel]
        s_dl = dl[sel]
        per_sec = []
        for c in range(NSEC):
            m = s_sec == c
            lane = poss[k][c][s_dl[m]]
            o = np.lexsort((s_idx[m], lane))
            lane_s = lane[o]
            val_s = s_idx[m][o]
            # cumcount within lane
            startmask = np.ones(len(lane_s), dtype=bool)
            startmask[1:] = lane_s[1:] != lane_s[:-1]
            startpos = np.where(startmask, np.arange(len(lane_s)), 0)
            np.maximum.accumulate(startpos, out=startpos)
            slot = np.arange(len(lane_s)) - startpos
            w = lane_s // P
            p = lane_s % P
            arr = np.full(sec_tot[c] * P, ZIDX, dtype=np.int16)
            arr[(woffD[c, w] + slot) * P + p] = val_s.astype(np.int16)
            per_sec.append(arr)
        main_idx.append(per_sec)

    # pack main idx into one [128, TOT/16] tensor per core, call-ordered
    call_off = []  # col16 offset per call
    o16 = 0
    for (c, wlo, whi, n) in calls:
        call_off.append(o16)
        o16 += n // 16
    TOT16 = o16
    ixmain = np.zeros((N_CORES, P, TOT16), dtype=np.int16)
    for k in range(N_CORES):
        for j, (c, wlo, whi, n) in enumerate(calls):
            a = main_idx[k][c][woffD[c, wlo] * P: woffD[c, whi] * P]
            ixmain[k, :, call_off[j]: call_off[j] + n // 16] = _wrap_idx(a)

    # ---- combine gather idx: groups of up to 8 final windows, 4 sections
    NWG = 8
    groups = []
    w0 = 0
    while w0 < NW:
        groups.append((w0, min(w0 + NWG, NW)))
        w0 = min(w0 + NWG, NW)
    comb16 = sum((whi - wlo) * P * NSEC // 16 for (wlo, whi) in groups)
    ixcomb = np.zeros((N_CORES, P, comb16), dtype=np.int16)
    comb_off = []
    o16 = 0
    for (wlo, whi) in groups:
        offs = []
        for c in range(NSEC):
            offs.append(o16)
            o16 += (whi - wlo) * P // 16
        comb_off.append(offs)
    for k in range(N_CORES):
        for gi, (wlo, whi) in enumerate(groups):
            d = np.arange(wlo * P, whi * P)
            for c in range(NSEC):
                a = poss[k][c][d].astype(np.int16)
                ixcomb[k, :, comb_off[gi][c]: comb_off[gi][c] + len(a) // 16] = \
                    _wrap_idx(a)

    # ---- per-core tensors
    W1b = np.asarray(W1, dtype=np.float32).astype(BF16)
    W2b = np.asarray(W2, dtype=np.float32).astype(BF16)
    b1r = np.tile(np.asarray(b1, dtype=np.float32)[None, :], (P, 1))
    degtot = deg.sum(axis=1)  # [cores, NPAD]
    b2v = np.asarray(b2, dtype=np.float32)
    xf = np.asarray(x, dtype=np.float32)
    in_maps = []
    for k in range(N_CORES):
        xT = np.ascontiguousarray(xf[k * NP:(k + 1) * NP].T).astype(BF16)
        degb2 = (degtot[k][:, None] * b2v[None, :]).astype(np.float32)
        in_maps.append(dict(
            xT=xT, W1=W1b, b1r=b1r, W2=W2b, degb2=degb2,
            ixmain=ixmain[k], ixcomb=ixcomb[k],
        ))
    sched = dict(
        D=D, calls=calls, call_off=call_off, TOT16=TOT16, woffD=woffD,
        groups=groups, comb_off=comb_off, COMB16=comb16,
    )
    return sched, in_maps


# ---------------------------------------------------------------- device code

def build_program(cfg, sched):
    import concourse.bass as bass
    import concourse.bacc as bacc
    import concourse.mybir as mybir
    from concourse.tile import TileContext
    from concourse.masks import make_identity

    dt = mybir.dt
    F, H, C = cfg["F"], cfg["H"], cfg["C"]
    NP, NW, NPAD = cfg["NP"], cfg["NW"], cfg["NPAD"]
    OFF, SZ = cfg["OFF"], cfg["SZ"]
    D, calls, call_off = sched["D"], sched["calls"], sched["call_off"]
    woffD, groups, comb_off = sched["woffD"], sched["groups"], sched["comb_off"]
    KF = F // P
    NWG_MAX = max(whi - wlo for (wlo, whi) in groups)

    nc = bacc.Bacc(
        "TRN2", target_bir_lowering=False, debug=False, num_devices=N_CORES,
        num_swdge_queues=4,
    )
    xT = nc.declare_dram_parameter("xT", [F, NP], dt.bfloat16, isOutput=False)
    W1p = nc.declare_dram_parameter("W1", [F, H], dt.bfloat16, isOutput=False)
    b1p = nc.declare_dram_parameter("b1r", [P, H], dt.float32, isOutput=False)
    W2p = nc.declare_dram_parameter("W2", [H, C], dt.bfloat16, isOutput=False)
    dgb = nc.declare_dram_parameter("degb2", [NPAD, C], dt.float32, isOutput=False)
    ixm = nc.declare_dram_parameter("ixmain", [P, sched["TOT16"]], dt.int16,
                                    isOutput=False)
    ixc = nc.declare_dram_parameter("ixcomb", [P, sched["COMB16"]], dt.int16,
                                    isOutput=False)
    outp = nc.declare_dram_parameter("out", [NPAD, C], dt.float32, isOutput=True)

    rg = [list(range(N_CORES))]

    with TileContext(nc) as tc:
        with (
            tc.tile_pool(name="const", bufs=1) as const,
            tc.tile_pool(name="dram", bufs=1, space="DRAM") as dram,
            tc.tile_pool(name="xp", bufs=3) as xp,
            tc.tile_pool(name="hp", bufs=3) as hp,
            tc.tile_pool(name="ixp", bufs=6) as ixp,
            tc.tile_pool(name="gp", bufs=6) as gpl,
            tc.tile_pool(name="pw", bufs=6) as pw,
            tc.tile_pool(name="cb", bufs=2) as cb,
            tc.tile_pool(name="sp", bufs=2) as sp,
            tc.tile_pool(name="fin", bufs=1) as fin,
            tc.tile_pool(name="ps", bufs=2, space="PSUM") as ps,
        ):
            # --- constants
            w1sb = const.tile([P, KF, H], dt.bfloat16)
            nc.sync.dma_start(out=w1sb[:], in_=W1p[:].rearrange("(c p) h -> p c h", p=P))
            w2sb = const.tile([H, C], dt.bfloat16)
            nc.sync.dma_start(out=w2sb[:], in_=W2p[:])
            b1sb = const.tile([P, H], dt.float32)
            nc.sync.dma_start(out=b1sb[:], in_=b1p[:])
            ident = const.tile([P, P], dt.bfloat16)
            make_identity(nc, ident[:])
            zrow = const.tile([1, H], dt.float32)
            nc.gpsimd.memset(zrow[:], 0.0)

            # --- DRAM
            h1k = dram.tile([NPAD, H], dt.float32)
            t2k = dram.tile([NPAD, H], dt.float32)
            tbl1 = [dram.tile([SECCAP, H], dt.float32, name=f"tbl1_{c}", tag=f"tbl1_{c}")
                    for c in range(NSEC)]
            tbl2 = [dram.tile([SECCAP, H], dt.float32, name=f"tbl2_{c}", tag=f"tbl2_{c}")
                    for c in range(NSEC)]
            part1 = dram.tile([NSEC, NPAD, H], dt.float32)
            part2 = dram.tile([NSEC, NPAD, H], dt.float32)

            for tbl in (tbl1, tbl2):
                for c in range(NSEC):
                    nc.sync.dma_start(
                        out=tbl[c][ZIDX: ZIDX + 1, :],
                        in_=zrow[:],
                    )

            # --- phase 1: h1 = x@W1 + b1 (fp32 rows), chunked AllGather
            xTr = xT[:].rearrange("(c p) n -> p c n", p=P)
            for ch in range(NSEC):
                lo, hi = OFF[ch], OFF[ch] + SZ[ch]
                nt0, nt1 = lo // P, (hi + P - 1) // P
                for nt in range(nt0, nt1):
                    r0, r1 = max(lo, nt * P), min(hi, (nt + 1) * P, NP)
                    if r1 <= r0:
                        # rows beyond NP: nothing to compute (padding rows)
                        continue
                    cs = r1 - r0
                    xt = xp.tile([P, KF, P], dt.bfloat16, tag="xt")
                    nc.sync.dma_start(out=xt[:, :, :cs], in_=xTr[:, :, r0:r1])
                    ph = ps.tile([P, H], dt.float32, tag="ph")
                    for kf in range(KF):
                        nc.tensor.matmul(
                            out=ph[:cs, :], lhsT=xt[:, kf, :cs], rhs=w1sb[:, kf, :],
                            start=(kf == 0), stop=(kf == KF - 1),
                        )
                    h1sb = hp.tile([P, H], dt.float32, tag="h1sb")
                    nc.vector.tensor_tensor(
                        out=h1sb[:cs, :], in0=ph[:cs, :], in1=b1sb[:cs, :],
                        op=mybir.AluOpType.add,
                    )
                    nc.sync.dma_start(out=h1k[r0:r0 + cs, :], in_=h1sb[:cs, :])
                nc.gpsimd.collective_compute(
                    "AllGather", mybir.AluOpType.bypass, replica_groups=rg,
                    ins=[h1k[lo:hi, :]],
                    outs=[tbl1[ch][: N_CORES * SZ[ch], :]],
                )

            # --- aggregation machinery (used for both layers)
            def run_layer(tbl, part):
                # main gathers + per-window reduce -> partials
                stage = {}  # section -> (tile, w_start, count)
                qn = 0
                for j, (c, wlo, whi, n) in enumerate(calls):
                    B = n // P
                    ix = ixp.tile([P, MAXIDX // 16], dt.int16, tag="ix")
                    nc.sync.dma_start(
                        out=ix[:, : n // 16],
                        in_=ixm[:, call_off[j]: call_off[j] + n // 16],
                    )
                    gt = gpl.tile([P, MAXIDX // P, H], dt.float32, tag="gt")
                    nc.gpsimd.dma_gather(
                        out_ap=gt[:, :B, :],
                        in_ap=tbl[c][:],
                        idxs_ap=ix[:, : n // 16],
                        num_idxs=n, num_idxs_reg=n, elem_size=H,
                        single_packet=False, queue_num=qn,
                    )
                    qn = (qn + 1) % 4
                    for w in range(wlo, whi):
                        b0 = int(woffD[c, w] - woffD[c, wlo])
                        dw = int(D[c, w])
                        if c not in stage or stage[c][1] + stage[c][2] != w \
                                or stage[c][2] == 8:
                            if c in stage:
                                _flush_stage(part, c, stage)
                            stage[c] = [pw.tile([P, 8, H], dt.float32, tag="st",
                                                name="st"),
                                        w, 0]
                        st, wst, cnt = stage[c]
                        nc.vector.tensor_reduce(
                            out=st[:, cnt, :],
                            in_=gt[:, b0:b0 + dw, :].rearrange("p b h -> p h b"),
                            axis=mybir.AxisListType.X,
                            op=mybir.AluOpType.add,
                        )
                        stage[c][2] += 1
                for c in list(stage.keys()):
                    _flush_stage(part, c, stage)

            def _flush_stage(part, c, stage):
                st, wst, cnt = stage.pop(c)
                nc.sync.dma_start(
                    out=part[c, wst * P:(wst + cnt) * P, :]
                        .rearrange("(w p) h -> p w h", p=P),
                    in_=st[:, :cnt, :],
                )

            def combine(part, gi):
                wlo, whi = groups[gi]
                nwg = whi - wlo
                big = cb.tile([P, NSEC, NWG_MAX, H], dt.float32, tag="big")
                for c in range(NSEC):
                    n = nwg * P
                    ix = ixp.tile([P, NWG_MAX * P // 16], dt.int16, tag="ixc")
                    nc.sync.dma_start(
                        out=ix[:, : n // 16],
                        in_=ixc[:, comb_off[gi][c]: comb_off[gi][c] + n // 16],
                    )
                    nc.gpsimd.dma_gather(
                        out_ap=big[:, c, :nwg, :],
                        in_ap=part[c],
                        idxs_ap=ix[:, : n // 16],
                        num_idxs=n, num_idxs_reg=n, elem_size=H,
                        single_packet=False, queue_num=c,
                    )
                t01 = sp.tile([P, NWG_MAX, H], dt.float32, tag="t01")
                nc.vector.tensor_tensor(out=t01[:, :nwg, :], in0=big[:, 0, :nwg, :],
                                        in1=big[:, 1, :nwg, :], op=mybir.AluOpType.add)
                t23 = sp.tile([P, NWG_MAX, H], dt.float32, tag="t23")
                nc.vector.tensor_tensor(out=t23[:, :nwg, :], in0=big[:, 2, :nwg, :],
                                        in1=big[:, 3, :nwg, :], op=mybir.AluOpType.add)
                f = sp.tile([P, NWG_MAX, H], dt.float32, tag="f")
                nc.vector.tensor_tensor(out=f[:, :nwg, :], in0=t01[:, :nwg, :],
                                        in1=t23[:, :nwg, :], op=mybir.AluOpType.add)
                return f, nwg

            # === layer 1 ===
            run_layer(tbl1, part1)

            # chunk-boundary bookkeeping for AllGather #2
            ch_after = {}
            for ch in range(NSEC):
                end = OFF[ch] + SZ[ch]
                for gi, (wlo, whi) in enumerate(groups):
                    if whi * P >= end:
                        ch_after.setdefault(gi, []).append(ch)
                        break

            for gi, (wlo, whi) in enumerate(groups):
                f, nwg = combine(part1, gi)
                # elu(f) = max(f,0) + exp(min(f,0)) - 1
                m = sp.tile([P, NWG_MAX, H], dt.float32, tag="m")
                nc.vector.tensor_scalar_min(out=m[:, :nwg, :], in0=f[:, :nwg, :],
                                            scalar1=0.0)
                e = sp.tile([P, NWG_MAX, H], dt.float32, tag="e")
                nc.scalar.activation(e[:, :nwg, :], m[:, :nwg, :],
                                     mybir.ActivationFunctionType.Exp)
                g1 = sp.tile([P, NWG_MAX, H], dt.float32, tag="g1")
                nc.vector.scalar_tensor_tensor(
                    out=g1[:, :nwg, :], in0=f[:, :nwg, :], scalar=0.0,
                    in1=e[:, :nwg, :],
                    op0=mybir.AluOpType.max, op1=mybir.AluOpType.add,
                )
                g2t = sp.tile([P, NWG_MAX, H], dt.float32, tag="g2t")
                nc.vector.tensor_scalar(
                    out=g2t[:, :nwg, :], in0=g1[:, :nwg, :], scalar1=-1.0,
                    scalar2=0.0, op0=mybir.AluOpType.add, op1=mybir.AluOpType.add,
                )
                nc.sync.dma_start(
                    out=t2k[wlo * P: whi * P, :].rearrange("(w p) h -> p w h", p=P),
                    in_=g2t[:, :nwg, :],
                )
                for ch in ch_after.get(gi, []):
                    lo, hi = OFF[ch], OFF[ch] + SZ[ch]
                    nc.gpsimd.collective_compute(
                        "AllGather", mybir.AluOpType.bypass, replica_groups=rg,
                        ins=[t2k[lo:hi, :]],
                        outs=[tbl2[ch][: N_CORES * SZ[ch], :]],
                    )

            # === layer 2 ===
            run_layer(tbl2, part2)

            t2f = fin.tile([P, NW, C], dt.float32)
            nmt = fin.tile([P, NW], dt.float32)
            sst = fin.tile([P, NW], dt.float32)
            for gi, (wlo, whi) in enumerate(groups):
                f, nwg = combine(part2, gi)
                db = sp.tile([P, NWG_MAX, C], dt.float32, tag="db")
                nc.sync.dma_start(
                    out=db[:, :nwg, :],
                    in_=dgb[wlo * P: whi * P, :].rearrange("(w p) c -> p w c", p=P),
                )
                for w in range(wlo, whi):
                    fb = sp.tile([P, H], dt.bfloat16, tag="fb")
                    nc.vector.tensor_copy(out=fb[:], in_=f[:, w - wlo, :])
                    tr = ps.tile([H, P], dt.bfloat16, tag="tr")
                    nc.tensor.transpose(out=tr[:], in_=fb[:], identity=ident[:])
                    trsb = sp.tile([H, P], dt.bfloat16, tag="trsb")
                    nc.vector.tensor_copy(out=trsb[:], in_=tr[:])
                    t2p = ps.tile([P, C], dt.float32, tag="t2p")
                    nc.tensor.matmul(out=t2p[:], lhsT=trsb[:], rhs=w2sb[:],
                                     start=True, stop=True)
                    nc.vector.tensor_tensor(
                        out=t2f[:, w, :], in0=t2p[:], in1=db[:, w - wlo, :],
                        op=mybir.AluOpType.add,
                    )
                    nc.vector.tensor_reduce(
                        out=nmt[:, w: w + 1], in_=t2f[:, w, :],
                        axis=mybir.AxisListType.X,
                        op=mybir.AluOpType.max, negate=True,
                    )
                    sc = sp.tile([P, C], dt.float32, tag="sc")
                    nc.scalar.activation(
                        sc[:], t2f[:, w, :], mybir.ActivationFunctionType.Exp,
                        bias=nmt[:, w: w + 1], accum_out=sst[:, w: w + 1],
                    )
            lnt = fin.tile([P, NW], dt.float32)
            nc.scalar.activation(lnt[:], sst[:], mybir.ActivationFunctionType.Ln)
            for gi, (wlo, whi) in enumerate(groups):
                nwg = whi - wlo
                ob = sp.tile([P, NWG_MAX, C], dt.float32, tag="ob")
                for w in range(wlo, whi):
                    nc.vector.tensor_scalar(
                        out=ob[:, w - wlo, :], in0=t2f[:, w, :],
                        scalar1=nmt[:, w: w + 1], scalar2=lnt[:, w: w + 1],
                        op0=mybir.AluOpType.add, op1=mybir.AluOpType.subtract,
                    )
                nc.sync.dma_start(
                    out=outp[wlo * P: whi * P, :].rearrange("(w p) c -> p w c", p=P),
                    in_=ob[:, :nwg, :],
                )

    nc.compile()
    return nc


# ---------------------------------------------------------------- entry point

LAST_RESULT = {}


def _run(cfg, x, edge_index, W1, b1, W2, b2, trace=False):
    from concourse.bass_utils import run_bass_kernel_spmd

    sched, in_maps = host_prep(cfg, x, edge_index, W1, b1, W2, b2)
    nc = build_program(cfg, sched)
    res = run_bass_kernel_spmd(nc, in_maps, list(range(N_CORES)), trace=trace)
    LAST_RESULT["exec_time_ns"] = res.exec_time_ns
    LAST_RESULT["mean_exec_time_ns"] = res.mean_exec_time_ns
    N, NP, C = cfg["N"], cfg["NP"], cfg["C"]
    full = np.empty((N, C), dtype=np.float32)
    for k in range(N_CORES):
        outk = np.asarray(res.results[k]["out"], dtype=np.float32)
        full[k * NP:(k + 1) * NP] = outk[:NP]
    return full


def kernel(x, edge_index, W1, b1, W2, b2):
    trace = bool(int(os.environ.get("GNN_TRACE", "0")))
    return _run(FULL_CFG, x, edge_index, W1, b1, W2, b2, trace=trace)
